# revision 26
# baseline (speedup 1.0000x reference)
"""Trainium2 Bass kernel for nn_MultiHeadAttention (channel-attention block).

Math per batch (X* = reshape(*, [C,P]), P=4096, C=128, D=512, 8 heads x 64):
  Q^T = Wq^T Xq^T, K^T = Wk^T Xk^T   (computed directly transposed, fp8 DR)
  V   = Xv Wv^T                      (bf16)
  per head: e = exp(Q_h K_h^T * esc); O_h = (e / rowsum(e)) V_h
  O = silu(O); xhat = (O - mean)/(unbiased_std + eps)   (LN affine folded
  into wfc_eff = w_fc * ln_gamma and veff = v + w_fc @ ln_beta)
  out_pre = veff + xhat @ wfc_eff^T
  out = BatchNorm2d(out_pre), batch stats over (b,h,w)

BN statistics are computed BEFORE the fc matmul via the decomposition
  sum_p out   = sum_p veff + xhat . wsum
  sum_p out^2 = sum_p veff^2 + 2 xhat . (v @ wfc_eff + vwc) + xhat . (xhat G)
with G = wfc_eff^T wfc_eff, wsum = sum_p wfc_eff, vwc = bias_fc @ wfc_eff
precomputed on host and VW = v @ wfc_eff accumulated on device during the
DMA-bound load phase.  The 1KB AllReduce of the stats therefore overlaps the
fc matmul instead of being a serial ~38us tail, and the BN affine is fused
into the fc epilogue.

Sharding: data-parallel over batch, 2 batches per core on 8 cores; BN stats
combined with a [128,2] AllReduce.

Dtypes: fp8e4 (x256 pre-scale, folded into the exp scale) for the Q/K path
and (x32) for the VW stats matmul -- both DoubleRow at 2x PE rate; bf16 for
the V / fc / residual path (the LN amplifies O errors ~20x there, fp8 fails
the 2e-2 gate; verified numerically on host).
"""

import os
from contextlib import ExitStack

import ml_dtypes
import numpy as np

import concourse.mybir as mybir
import concourse.tile as tile
from concourse import bacc
from concourse.bass_utils import run_bass_kernel_spmd
from concourse.masks import make_identity

# ---- problem constants (hardcoded per contract) ----
B, C, HH, WW = 16, 128, 64, 64
P = HH * WW           # 4096
NH, LD = 8, 64
D = NH * LD           # 512
N_CORES = 8
BPC = B // N_CORES    # 2 batches per core
NC4 = 16              # 256-row contraction chunks (DoubleRow)
LN_EPS = 1e-6
BN_EPS = 1e-5
F32 = mybir.dt.float32
BF16 = mybir.dt.bfloat16
FP8 = mybir.dt.float8e4
DR = mybir.MatmulPerfMode.DoubleRow

SC_QK = 256.0         # fp8 pre-scale for wq/wk (keeps them out of subnormals)
SC_VW = 32.0          # fp8 pre-scale for wfn in the VW stats matmul

MODE = "v2"           # printed by test.py
_ENV_KEY = lambda: (os.environ.get("V2_WARMAR", "1"), os.environ.get("V2_SKIP_COLL", "0"))
_BUILD_CACHE: dict = {}
LAST_RESULTS = None


def _emit(ctx, nc, tc, io, temp):
    AF = mybir.ActivationFunctionType
    ALU = mybir.AluOpType
    esc = 1.0 / (SC_QK * SC_QK * temp)   # exp arg: undo fp8 pre-scale + temperature

    consts = ctx.enter_context(tc.tile_pool(name="consts", bufs=1))
    wpool = ctx.enter_context(tc.tile_pool(name="wpool", bufs=3))
    apool = ctx.enter_context(tc.tile_pool(name="apool", bufs=3))
    res = ctx.enter_context(tc.tile_pool(name="res", bufs=1))    # resident
    sb = ctx.enter_context(tc.tile_pool(name="sb", bufs=2))
    small = ctx.enter_context(tc.tile_pool(name="small", bufs=6))
    stg = ctx.enter_context(tc.tile_pool(name="stg", bufs=4))
    tpool = ctx.enter_context(tc.tile_pool(name="tp", bufs=4))
    dram = ctx.enter_context(tc.tile_pool(name="dram", bufs=1, space="DRAM"))

    # identity for PE transposes (bf16)
    ident_f = consts.tile([128, 128], F32, tag="identf", name="identf")
    make_identity(nc, ident_f)
    ident = consts.tile([128, 128], BF16, tag="ident", name="ident")
    nc.vector.tensor_copy(out=ident, in_=ident_f)

    # ---- warm-up collective: absorbs the CC entry barrier early ----
    if os.environ.get("V2_WARMAR", "1") == "1":
        cw_in = dram.tile([128, 1], F32, tag="cw_in", name="cw_in")
        cw_out = dram.tile([128, 1], F32, tag="cw_out", name="cw_out")
        warm_sb = consts.tile([128, 1], F32, tag="warm_sb", name="warm_sb")
        nc.vector.memset(warm_sb, 0.0)
        nc.gpsimd.dma_start(out=cw_in[:, :], in_=warm_sb)
        nc.gpsimd.collective_compute(
            "AllReduce", ALU.add, replica_groups=[list(range(N_CORES))],
            ins=[cw_in.opt()], outs=[cw_out.opt()])

    bng = consts.tile([128, 1], F32, tag="bng", name="bng")
    bnb = consts.tile([128, 1], F32, tag="bnb", name="bnb")
    svin = consts.tile([128, 2 * BPC], F32, tag="svin", name="svin")
    nc.gpsimd.dma_start(out=bng, in_=io["bng"][:, :])
    nc.gpsimd.dma_start(out=bnb, in_=io["bnb"][:, :])
    nc.gpsimd.dma_start(out=svin, in_=io["svin"][:, :])

    # ---- PSUM: warm-up transpose in a throwaway pool ----
    with tc.tile_pool(name="ps_wm", bufs=1, space="PSUM") as pw:
        warm = pw.tile([128, 128], BF16, tag="warmt", name="warmt")
        nc.tensor.transpose(warm[:, :], ident[:, :], ident[:, :])

    # ---- resident SBUF tensors (issue queues chosen for criticality) ----
    # vT16: gpsimd queue, early -- needed by the V projection.
    vT16_sb = []
    for b in range(BPC):
        t16 = res.tile([128, 32, 128], BF16, tag=f"vT16_{b}", name=f"vT16_{b}")
        for g in range(2):
            nc.gpsimd.dma_start(out=t16[:, 16 * g:16 * g + 16],
                                in_=io["vT16"][b, :, 16 * g:16 * g + 16])
        vT16_sb.append(t16)

    # VW accumulators: outermost long-lived PSUM pool (2 banks)
    ps_vw = ctx.enter_context(tc.tile_pool(name="ps_vw", bufs=1, space="PSUM"))
    VWp = [ps_vw.tile([128, D], F32, tag=f"VWp{b}", name=f"VWp{b}") for b in range(BPC)]

    ps_qkv = tc.tile_pool(name="ps_qkv", bufs=1, space="PSUM")
    pa = ps_qkv.__enter__()
    QTp = [pa.tile([128, 4, 128], F32, tag=f"QTp{b}", name=f"QTp{b}") for b in range(BPC)]
    KTp = [pa.tile([128, 4, 128], F32, tag=f"KTp{b}", name=f"KTp{b}") for b in range(BPC)]
    Vp = [pa.tile([128, D], F32, tag=f"Vp{b}", name=f"Vp{b}") for b in range(BPC)]

    # ---- phase A1: Q^T/K^T projections (fp8 DR) + V (bf16), streaming ----
    for pc in range(NC4):
        wq_c = wpool.tile([128, 2, D], FP8, tag="wq_c", name="wq_c")
        wk_c = wpool.tile([128, 2, D], FP8, tag="wk_c", name="wk_c")
        nc.sync.dma_start(out=wq_c, in_=io["wq8"][pc])
        nc.sync.dma_start(out=wk_c, in_=io["wk8"][pc])
        qcs, kcs = [], []
        for b in range(BPC):
            qc = apool.tile([128, 2, 128], FP8, tag=f"qc{b}", name=f"qc{b}")
            kc = apool.tile([128, 2, 128], FP8, tag=f"kc{b}", name=f"kc{b}")
            nc.scalar.dma_start(out=qc, in_=io["qT8"][b, pc])
            nc.scalar.dma_start(out=kc, in_=io["kT8"][b, pc])
            qcs.append(qc); kcs.append(kc)
        for b in range(BPC):
            for db in range(4):
                # one PSUM accumulation group per bank: start only on the
                # first write into the bank, stop on the very last
                st = pc == 0 and db == 0
                sp = pc == NC4 - 1 and db == 3
                nc.tensor.matmul(QTp[b][:, db, :], wq_c[:, :, db * 128:(db + 1) * 128],
                                 qcs[b][:, :, :], start=st, stop=sp, perf_mode=DR)
                nc.tensor.matmul(KTp[b][:, db, :], wk_c[:, :, db * 128:(db + 1) * 128],
                                 kcs[b][:, :, :], start=st, stop=sp, perf_mode=DR)
        # V: two 128-row chunks per DR chunk (bf16, classic orientation)
        for half in range(2):
            pc2 = 2 * pc + half
            wv_c = wpool.tile([128, D], BF16, tag="wv_c", name="wv_c")
            nc.scalar.dma_start(out=wv_c, in_=io["wv16"][pc2])
            for b in range(BPC):
                nc.tensor.matmul(Vp[b][:, :], vT16_sb[b][:, pc2, :], wv_c[:, :],
                                 start=pc2 == 0, stop=pc2 == 31)

    # marker: lands after the whole wq/wk stream on the sync queue; gates the
    # gpsimd residual stream below so it doesn't steal stage-1 bandwidth
    marker = consts.tile([128, 1], F32, tag="marker", name="marker")
    nc.sync.dma_start(out=marker, in_=io["bng"][:, :])

    # VW inputs behind the wq/wk stream on sync (needed ~mid-kernel)
    vT8_sb = []
    for b in range(BPC):
        t8 = res.tile([128, NC4, 2, 128], FP8, tag=f"vT8_{b}", name=f"vT8_{b}")
        nc.sync.dma_start(out=t8[:, 0:8], in_=io["vT8"][b, :, 0:8])
        nc.sync.dma_start(out=t8[:, 8:16], in_=io["vT8"][b, :, 8:16])
        vT8_sb.append(t8)
    wfn_sb = res.tile([128, NC4, 2, D], FP8, tag="wfn", name="wfn")
    for g in range(4):
        nc.sync.dma_start(out=wfn_sb[:, 4 * g:4 * g + 4], in_=io["wfn"][:, 4 * g:4 * g + 4])
    G_sb = res.tile([128, 4, D], BF16, tag="G", name="G")
    nc.scalar.dma_start(out=G_sb[:, :], in_=io["G"][:, :])
    wv2 = res.tile([128, 4, 2], BF16, tag="wv2", name="wv2")
    nc.scalar.dma_start(out=wv2[:, :], in_=io["wv2"][:, :])

    # ---- phase A2: VW = (v @ wfc_eff) * SC_VW via fp8 DR, accumulating ----
    for pc in range(NC4):
        for b in range(BPC):
            nc.tensor.matmul(VWp[b][:, :], vT8_sb[b][:, pc, :, :], wfn_sb[:, pc, :, :],
                             start=pc == 0, stop=pc == NC4 - 1, perf_mode=DR)

    # fc weights: scalar queue, behind the act/wv16 stream
    wfcT = res.tile([128, 4, P], BF16, tag="wfcT", name="wfcT")
    for dc in range(4):
        nc.scalar.dma_start(out=wfcT[:, dc], in_=io["wfcT"][dc])

    # ---- evacuate QT/KT/V to SBUF (bf16), then free those PSUM banks ----
    qkv_sb = []
    for b in range(BPC):
        QT_sb = sb.tile([128, 4, 128], BF16, tag="QT_sb", name="QT_sb")
        KT_sb = sb.tile([128, 4, 128], BF16, tag="KT_sb", name="KT_sb")
        V_sb = sb.tile([128, D], BF16, tag="V_sb", name="V_sb")
        nc.vector.tensor_copy(out=QT_sb, in_=QTp[b][:, :, :])
        nc.scalar.copy(out=KT_sb, in_=KTp[b][:, :, :])
        nc.vector.tensor_copy(out=V_sb, in_=Vp[b][:, :])
        qkv_sb.append((QT_sb, KT_sb, V_sb))
    ps_qkv.__exit__(None, None, None)

    # residual: gpsimd queue, gated behind the stage-1 marker
    mdump = dram.tile([128, 1], F32, tag="mdump", name="mdump")
    nc.gpsimd.dma_start(out=mdump[:, :], in_=marker)
    veff_sb = []
    for b in range(BPC):
        t = res.tile([128, P], BF16, tag=f"veff{b}", name=f"veff{b}")
        nc.gpsimd.dma_start(out=t[:, 0:2048], in_=io["veff"][b, :, 0:2048])
        nc.gpsimd.dma_start(out=t[:, 2048:4096], in_=io["veff"][b, :, 2048:4096])
        veff_sb.append(t)

    # attention-era PSUM: one f32 bank (3 S slots + 2 a12 slots), one bf16
    # transpose bank (8 slots), one O bank, 2 fc banks (also used for Zp).
    # PSUM reserves a full 2KB bank per tag-buffer, so slots are hand-sliced.
    ps_at = ctx.enter_context(tc.tile_pool(name="ps_at", bufs=1, space="PSUM"))
    Sbank = ps_at.tile([128, 4, 128], F32, tag="Sbank", name="Sbank")
    Tbank = ps_at.tile([128, 8, 128], BF16, tag="Tbank", name="Tbank")
    Obank = [ps_at.tile([128, D], F32, tag=f"Ob{b}", name=f"Ob{b}") for b in range(BPC)]
    ps_fc = ctx.enter_context(tc.tile_pool(name="ps_fc", bufs=2, space="PSUM"))
    tslot = [0]

    def tslot_next():
        s = tslot[0] % 8
        tslot[0] += 1
        return s

    # ---- attention + silu + LN + stats per batch ----
    xTs = []
    cin_sb = small.tile([128, 2], F32, tag="cin_sb", name="cin_sb")
    s1l = [small.tile([128, 1], F32, tag=f"s1l{b}", name=f"s1l{b}") for b in range(BPC)]
    s2l = [small.tile([128, 1], F32, tag=f"s2l{b}", name=f"s2l{b}") for b in range(BPC)]
    for b in range(BPC):
        QT_sb, KT_sb, V_sb = qkv_sb[b]
        Opsum = Obank[b]
        Osc = sb.tile([128, D], F32, tag="Osc", name="Osc")
        for h in range(NH):
            po = (h % 2) * 64
            dc = h // 2
            S = Sbank[:, h % 3, :]
            nc.tensor.matmul(S, QT_sb[po:po + 64, dc, :], KT_sb[po:po + 64, dc, :],
                             start=True, stop=True)
            e_f = sb.tile([128, 128], BF16, tag="e_f", name="e_f")
            lsum = small.tile([128, 1], F32, tag="lsum", name="lsum")
            nc.scalar.activation(out=e_f, in_=S, func=AF.Exp, scale=esc,
                                 accum_out=lsum)
            rs = small.tile([128, 1], F32, tag="rs", name="rs")
            nc.vector.reciprocal(rs, lsum)
            tpa = Tbank[:, tslot_next(), :]
            nc.tensor.transpose(tpa, e_f[:, :], ident[:, :])
            aT = sb.tile([128, 128], BF16, tag="aT", name="aT")
            nc.scalar.copy(out=aT, in_=tpa)
            nc.tensor.matmul(Opsum[:, h * 64:(h + 1) * 64], aT[:, :],
                             V_sb[:, h * 64:(h + 1) * 64], start=True, stop=True)
            nc.vector.tensor_scalar_mul(out=Osc[:, h * 64:(h + 1) * 64],
                                        in0=Opsum[:, h * 64:(h + 1) * 64], scalar1=rs)

        # silu + layernorm (affine folded into wfc_eff/veff on host)
        sg = sb.tile([128, D], F32, tag="sg", name="sg")
        nc.scalar.activation(out=sg, in_=Osc, func=AF.Sigmoid)
        Osw = sb.tile([128, D], F32, tag="Osw", name="Osw")
        nc.vector.tensor_mul(out=Osw, in0=Osc, in1=sg)
        st6 = small.tile([128, 6], F32, tag="st6", name="st6")
        nc.vector.bn_stats(out=st6, in_=Osw)
        mv = small.tile([128, 2], F32, tag="mv", name="mv")
        nc.vector.bn_aggr(out=mv, in_=st6)
        sd = small.tile([128, 1], F32, tag="sd", name="sd")
        nc.scalar.activation(out=sd, in_=mv[:, 1:2], func=AF.Sqrt,
                             scale=float(D) / (D - 1))
        nc.vector.tensor_scalar_add(out=sd, in0=sd, scalar1=LN_EPS)
        rstd = small.tile([128, 1], F32, tag="rstd", name="rstd")
        nc.vector.reciprocal(rstd, sd)
        xhat = sb.tile([128, D], BF16, tag="xhat", name="xhat")
        nc.vector.tensor_scalar(out=xhat, in0=Osw, scalar1=mv[:, 0:1], scalar2=rstd,
                                op0=ALU.subtract, op1=ALU.mult)
        xT = sb.tile([128, 4, 128], BF16, tag="xT", name="xT")
        for dc in range(4):
            tp = Tbank[:, tslot_next(), :]
            nc.tensor.transpose(tp, xhat[:, dc * 128:(dc + 1) * 128], ident[:, :])
            nc.vector.tensor_copy(out=xT[:, dc, :], in_=tp)
        xTs.append(xT)

        # stats: S1 = sv1 + xhat.wsum ; S2 = sv2 + (2/SC)xhat.VW + 2 xhat.vwc + xhat.(xhat G)
        Zp = ps_fc.tile([128, D], F32, tag="O2", name="Zp")
        a12 = Sbank[:, 3, 2 * b:2 * b + 2]
        for dc in range(4):
            nc.tensor.matmul(Zp[:, :], xT[:, dc, :], G_sb[:, dc, :],
                             start=dc == 0, stop=dc == 3)
            nc.tensor.matmul(a12, xT[:, dc, :], wv2[:, dc, :],
                             start=dc == 0, stop=dc == 3)
        AX = mybir.AxisListType
        j1 = tpool.tile([128, D], BF16, tag="junk", name="junk1")
        j2 = tpool.tile([128, D], BF16, tag="junk", name="junk2")
        nc.vector.tensor_mul(out=j1, in0=xhat, in1=VWp[b][:, :])
        nc.vector.tensor_mul(out=j2, in0=xhat, in1=Zp[:, :])
        r1 = small.tile([128, 1], F32, tag="r1", name="r1")
        r2 = small.tile([128, 1], F32, tag="r2", name="r2")
        nc.vector.reduce_sum(r1, j1, axis=AX.X)
        nc.vector.reduce_sum(r2, j2, axis=AX.X)
        # S2 = sv2 + (2/SC_VW) r1 + r2 + a12[:,1]
        s2a = small.tile([128, 1], F32, tag="s2a", name="s2a")
        nc.vector.tensor_scalar(out=s2a, in0=r1, scalar1=2.0 / SC_VW,
                                scalar2=svin[:, 2 * b + 1:2 * b + 2],
                                op0=ALU.mult, op1=ALU.add)
        s2b = small.tile([128, 1], F32, tag="s2b", name="s2b")
        nc.vector.tensor_add(out=s2b, in0=s2a, in1=r2)
        nc.vector.tensor_add(out=s2l[b], in0=s2b, in1=a12[:, 1:2])
        nc.vector.tensor_add(out=s1l[b], in0=svin[:, 2 * b:2 * b + 1], in1=a12[:, 0:1])
    nc.vector.tensor_add(out=cin_sb[:, 0:1], in0=s1l[0], in1=s1l[1])
    nc.vector.tensor_add(out=cin_sb[:, 1:2], in0=s2l[0], in1=s2l[1])

    # ---- stats AllReduce (overlaps the fc phase below) ----
    cin = dram.tile([128, 2], F32, tag="cin", name="cin")
    cout = dram.tile([128, 2], F32, tag="cout", name="cout")
    nc.gpsimd.dma_start(out=cin[:, :], in_=cin_sb)
    if os.environ.get("V2_SKIP_COLL", "0") == "1":
        nc.gpsimd.dma_start(out=cout[:, :], in_=cin[:, :])
    else:
        nc.gpsimd.collective_compute(
            "AllReduce", ALU.add, replica_groups=[list(range(N_CORES))],
            ins=[cin.opt()], outs=[cout.opt()])
    red = small.tile([128, 2], F32, tag="red", name="red")
    nc.gpsimd.dma_start(out=red[:, :], in_=cout[:, :])

    # ---- fc matmuls + residual add (not AR-gated) ----
    tsegs = []
    for pt in range(8):
        for b in range(BPC):
            O2 = ps_fc.tile([128, 512], F32, tag="O2", name="O2")
            for dc in range(4):
                nc.tensor.matmul(O2[:, :], xTs[b][:, dc, :],
                                 wfcT[:, dc, pt * 512:(pt + 1) * 512],
                                 start=dc == 0, stop=dc == 3)
            tseg = tpool.tile([128, 512], BF16, tag="tseg", name="tseg")
            nc.vector.tensor_add(out=tseg, in0=veff_sb[b][:, pt * 512:(pt + 1) * 512],
                                 in1=O2[:, :])
            tsegs.append((b, pt, tseg))

    # ---- post-AR: BN affine factors ----
    inv_n = 1.0 / float(B * P)
    mean = small.tile([128, 1], F32, tag="mean", name="mean")
    nc.scalar.mul(out=mean, in_=red[:, 0:1], mul=inv_n)
    ex2 = small.tile([128, 1], F32, tag="ex2", name="ex2")
    nc.scalar.mul(out=ex2, in_=red[:, 1:2], mul=inv_n)
    msq = small.tile([128, 1], F32, tag="msq", name="msq")
    nc.vector.tensor_mul(out=msq, in0=mean, in1=mean)
    var = small.tile([128, 1], F32, tag="var", name="var")
    nc.vector.tensor_sub(out=var, in0=ex2, in1=msq)
    epsbn = consts.tile([128, 1], F32, tag="epsbn", name="epsbn")
    nc.vector.memset(epsbn, BN_EPS)
    sdv = small.tile([128, 1], F32, tag="sdv", name="sdv")
    nc.scalar.activation(out=sdv, in_=var, func=AF.Sqrt, bias=epsbn)
    invs = small.tile([128, 1], F32, tag="invs", name="invs")
    nc.vector.reciprocal(invs, sdv)
    scl = small.tile([128, 1], F32, tag="scl", name="scl")
    nc.vector.tensor_mul(out=scl, in0=bng, in1=invs)
    tmp = small.tile([128, 1], F32, tag="tmp", name="tmp")
    nc.vector.tensor_mul(out=tmp, in0=mean, in1=scl)
    shf = small.tile([128, 1], F32, tag="shf", name="shf")
    nc.vector.tensor_sub(out=shf, in0=bnb, in1=tmp)

    # ---- AR-gated epilogue: scale+shift, split ACT/DVE, store ----
    for i, (b, pt, tseg) in enumerate(tsegs):
        stage = stg.tile([128, 512], BF16, tag="stage", name="stage")
        if i % 2 == 0:
            nc.scalar.activation(out=stage, in_=tseg, func=AF.Identity,
                                 bias=shf, scale=scl)
        else:
            nc.vector.tensor_scalar(out=stage, in0=tseg, scalar1=scl, scalar2=shf,
                                    op0=ALU.mult, op1=ALU.add)
        eng = nc.sync if i % 2 == 0 else nc.gpsimd
        eng.dma_start(out=io["out"][b, :, pt * 512:(pt + 1) * 512], in_=stage)


def _build(temp):
    key = (MODE, temp, _ENV_KEY())
    if key in _BUILD_CACHE:
        return _BUILD_CACHE[key]
    nc = bacc.Bacc("TRN2", target_bir_lowering=False, debug=False, num_devices=N_CORES)
    io = {
        "qT8": nc.dram_tensor("qT8", [BPC, NC4, 128, 2, 128], FP8, kind="ExternalInput").ap(),
        "kT8": nc.dram_tensor("kT8", [BPC, NC4, 128, 2, 128], FP8, kind="ExternalInput").ap(),
        "vT8": nc.dram_tensor("vT8", [BPC, 128, NC4, 2, 128], FP8, kind="ExternalInput").ap(),
        "vT16": nc.dram_tensor("vT16", [BPC, 128, 32, 128], BF16, kind="ExternalInput").ap(),
        "veff": nc.dram_tensor("veff", [BPC, C, P], BF16, kind="ExternalInput").ap(),
        "wq8": nc.dram_tensor("wq8", [NC4, 128, 2, D], FP8, kind="ExternalInput").ap(),
        "wk8": nc.dram_tensor("wk8", [NC4, 128, 2, D], FP8, kind="ExternalInput").ap(),
        "wv16": nc.dram_tensor("wv16", [32, 128, D], BF16, kind="ExternalInput").ap(),
        "wfn": nc.dram_tensor("wfn", [128, NC4, 2, D], FP8, kind="ExternalInput").ap(),
        "wfcT": nc.dram_tensor("wfcT", [4, 128, P], BF16, kind="ExternalInput").ap(),
        "G": nc.dram_tensor("G", [128, 4, D], BF16, kind="ExternalInput").ap(),
        "wv2": nc.dram_tensor("wv2", [128, 4, 2], BF16, kind="ExternalInput").ap(),
        "svin": nc.dram_tensor("svin", [C, 2 * BPC], F32, kind="ExternalInput").ap(),
        "bng": nc.dram_tensor("bng", [C, 1], F32, kind="ExternalInput").ap(),
        "bnb": nc.dram_tensor("bnb", [C, 1], F32, kind="ExternalInput").ap(),
        "out": nc.dram_tensor("out", [BPC, C, P], BF16, kind="ExternalOutput").ap(),
    }
    with tile.TileContext(nc) as tc, ExitStack() as ctx:
        _emit(ctx, nc, tc, io, temp)
    nc.compile()
    _BUILD_CACHE[key] = nc
    return nc


def _pack_dr(xT):
    # [P, M] -> [NC4, 128, 2, M]: DR chunk pc covers rows pc*256 + i*128 + r
    M = xT.shape[1]
    return np.ascontiguousarray(xT.reshape(NC4, 2, 128, M).transpose(0, 2, 1, 3))


def _f8(x):
    return np.ascontiguousarray(np.asarray(x, np.float32).astype(ml_dtypes.float8_e4m3))


def _b16(x):
    return np.ascontiguousarray(np.asarray(x, np.float32).astype(ml_dtypes.bfloat16))


def kernel(v, k, q, w_qs, w_ks, w_vs, w_fc, ln_gamma, ln_beta, temperature,
           bn_gamma, bn_beta, **_ignored):
    v = np.asarray(v, np.float32)
    k = np.asarray(k, np.float32)
    q = np.asarray(q, np.float32)
    w_qs = np.asarray(w_qs, np.float32)
    w_ks = np.asarray(w_ks, np.float32)
    w_vs = np.asarray(w_vs, np.float32)
    w_fc = np.asarray(w_fc, np.float32)
    ln_gamma = np.asarray(ln_gamma, np.float32)
    ln_beta = np.asarray(ln_beta, np.float32)
    temp = float(np.asarray(temperature))
    bn_gamma = np.asarray(bn_gamma, np.float32)
    bn_beta = np.asarray(bn_beta, np.float32)

    qf = q.reshape(B, C, P)
    kf = k.reshape(B, C, P)
    vf = v.reshape(B, C, P)

    # activations: [P, C] transposed, DR-packed for q/k; r-major for vT8/vT16
    qT8 = np.stack([_f8(_pack_dr(qf[b].T)) for b in range(B)])
    kT8 = np.stack([_f8(_pack_dr(kf[b].T)) for b in range(B)])
    # vT8 SBUF layout [r, pc, i, c]:
    vT8 = np.stack([_f8(vf[b].T.reshape(NC4, 2, 128, C).transpose(2, 0, 1, 3))
                    for b in range(B)])
    # vT16 SBUF layout [r, pc2, c]:
    vT16 = np.stack([_b16(vf[b].T.reshape(32, 128, C).transpose(1, 0, 2))
                     for b in range(B)])

    # weights
    wq8 = _f8(_pack_dr(w_qs.T * SC_QK))
    wk8 = _f8(_pack_dr(w_ks.T * SC_QK))
    wv16 = _b16(w_vs.T.reshape(32, 128, D))
    wfc_eff = (w_fc * ln_gamma[None, :]).astype(np.float32)       # [P, D]
    # wfn SBUF layout [r, pc, i, d]:
    wfn8 = _f8((wfc_eff * SC_VW).reshape(NC4, 2, 128, D).transpose(2, 0, 1, 3))
    wfcT16 = _b16(wfc_eff.T.reshape(4, 128, P))
    G = (wfc_eff.T.astype(np.float64) @ wfc_eff.astype(np.float64)).astype(np.float32)
    G16 = _b16(G.reshape(4, 128, D).transpose(1, 0, 2))           # [r, dc, d]
    bias_fc = (w_fc @ ln_beta).astype(np.float32)                 # [P]
    wsum = wfc_eff.sum(0)
    vwc = (bias_fc.astype(np.float64) @ wfc_eff.astype(np.float64)).astype(np.float32)
    wv2 = _b16(np.stack([wsum, 2.0 * vwc], axis=1)
               .reshape(4, 128, 2).transpose(1, 0, 2))            # [r, dc, j]

    veff_f = vf + bias_fc[None, None, :]
    veff = _b16(veff_f)
    # per-batch input-residual partial sums for the BN stats decomposition
    sv1 = veff_f.astype(np.float64).sum(-1)                       # [B, C]
    sv2 = (veff_f.astype(np.float64) ** 2).sum(-1)
    svin_all = np.stack([sv1, sv2], axis=-1).astype(np.float32)   # [B, C, 2]

    bng = np.ascontiguousarray(bn_gamma.reshape(C, 1))
    bnb = np.ascontiguousarray(bn_beta.reshape(C, 1))

    nc = _build(temp)
    in_maps = []
    for i in range(N_CORES):
        bs = slice(BPC * i, BPC * (i + 1))
        # svin layout: [c, 2*b + (0:sum, 1:sumsq)]
        svin = np.ascontiguousarray(
            svin_all[bs].transpose(1, 0, 2).reshape(C, 2 * BPC))
        in_maps.append({
            "qT8": qT8[bs], "kT8": kT8[bs], "vT8": vT8[bs], "vT16": vT16[bs],
            "veff": veff[bs], "wq8": wq8, "wk8": wk8, "wv16": wv16,
            "wfn": wfn8, "wfcT": wfcT16, "G": G16, "wv2": wv2,
            "svin": svin, "bng": bng, "bnb": bnb,
        })
    res = run_bass_kernel_spmd(nc, in_maps, core_ids=list(range(N_CORES)))
    global LAST_RESULTS
    LAST_RESULTS = res
    out = np.concatenate([np.asarray(res.results[i]["out"], np.float32)
                          for i in range(N_CORES)], axis=0)
    return out.reshape(B, C, HH, WW)


# revision 34
# speedup vs baseline: 1.1700x; 1.1700x over previous
"""Trainium2 Bass kernel for nn_MultiHeadAttention (channel-attention block).

Math per batch (X* = reshape(*, [C,P]), P=4096, C=128, D=512, 8 heads x 64):
  Q^T = Wq^T Xq^T, K^T = Wk^T Xk^T   (computed directly transposed, fp8 DR)
  V   = Xv Wv^T                      (bf16)
  per head: e = exp(Q_h K_h^T * esc); O_h = (e / rowsum(e)) V_h
  O = silu(O); xhat = (O - mean)/(unbiased_std + eps)   (LN affine folded
  into wfc_eff = w_fc * ln_gamma and veff = v + w_fc @ ln_beta)
  out_pre = veff + xhat @ wfc_eff^T
  out = BatchNorm2d(out_pre), batch stats over (b,h,w)

BN statistics are computed BEFORE the fc matmul via the decomposition
  sum_p out   = sum_p veff + xhat . wsum
  sum_p out^2 = sum_p veff^2 + 2 xhat . (v @ wfc_eff + vwc) + xhat . (xhat G)
with G = wfc_eff^T wfc_eff, wsum = sum_p wfc_eff, vwc = bias_fc @ wfc_eff
precomputed on host and VW = v @ wfc_eff accumulated on device during the
DMA-bound load phase.  The 1KB AllReduce of the stats therefore overlaps the
fc matmul instead of being a serial ~38us tail, and the BN affine is fused
into the fc epilogue.

Sharding: data-parallel over batch, 2 batches per core on 8 cores; BN stats
combined with a [128,2] AllReduce.

Dtypes: fp8e4 (x256 pre-scale, folded into the exp scale) for the Q/K path
and (x32) for the VW stats matmul -- both DoubleRow at 2x PE rate; bf16 for
the V / fc / residual path (the LN amplifies O errors ~20x there, fp8 fails
the 2e-2 gate; verified numerically on host).
"""

import os
from contextlib import ExitStack

import ml_dtypes
import numpy as np

import concourse.mybir as mybir
import concourse.tile as tile
from concourse import bacc
from concourse.bass_utils import run_bass_kernel_spmd
from concourse.masks import make_identity

# ---- problem constants (hardcoded per contract) ----
B, C, HH, WW = 16, 128, 64, 64
P = HH * WW           # 4096
NH, LD = 8, 64
D = NH * LD           # 512
N_CORES = 8
BPC = B // N_CORES    # 2 batches per core
NC4 = 16              # 256-row contraction chunks (DoubleRow)
LN_EPS = 1e-6
BN_EPS = 1e-5
F32 = mybir.dt.float32
BF16 = mybir.dt.bfloat16
FP8 = mybir.dt.float8e4
DR = mybir.MatmulPerfMode.DoubleRow

SC_QK = 256.0         # fp8 pre-scale for wq/wk (keeps them out of subnormals)
SC_VW = 32.0          # fp8 pre-scale for wfn in the VW stats matmul

MODE = "v2"           # printed by test.py
_ENV_KEY = lambda: (os.environ.get("V2_WARMAR", "1"), os.environ.get("V2_SKIP_COLL", "0"))
_BUILD_CACHE: dict = {}
LAST_RESULTS = None


def _emit(ctx, nc, tc, io, temp):
    AF = mybir.ActivationFunctionType
    ALU = mybir.AluOpType
    esc = 1.0 / (SC_QK * SC_QK * temp)   # exp arg: undo fp8 pre-scale + temperature

    consts = ctx.enter_context(tc.tile_pool(name="consts", bufs=1))
    wpool = ctx.enter_context(tc.tile_pool(name="wpool", bufs=2))
    apool = ctx.enter_context(tc.tile_pool(name="apool", bufs=1))
    res = ctx.enter_context(tc.tile_pool(name="res", bufs=1))    # resident
    sb = ctx.enter_context(tc.tile_pool(name="sb", bufs=2))
    small = ctx.enter_context(tc.tile_pool(name="small", bufs=6))
    stg = ctx.enter_context(tc.tile_pool(name="stg", bufs=4))
    tpool = ctx.enter_context(tc.tile_pool(name="tp", bufs=4))
    dram = ctx.enter_context(tc.tile_pool(name="dram", bufs=1, space="DRAM"))

    # identity for PE transposes (bf16)
    ident_f = consts.tile([128, 128], F32, tag="identf", name="identf")
    make_identity(nc, ident_f)
    ident = consts.tile([128, 128], BF16, tag="ident", name="ident")
    nc.vector.tensor_copy(out=ident, in_=ident_f)

    # ---- warm-up collective: absorbs the CC entry barrier early ----
    if os.environ.get("V2_WARMAR", "1") == "1":
        cw_in = dram.tile([128, 1], F32, tag="cw_in", name="cw_in")
        cw_out = dram.tile([128, 1], F32, tag="cw_out", name="cw_out")
        warm_sb = consts.tile([128, 1], F32, tag="warm_sb", name="warm_sb")
        nc.vector.memset(warm_sb, 0.0)
        nc.gpsimd.dma_start(out=cw_in[:, :], in_=warm_sb)
        nc.gpsimd.collective_compute(
            "AllReduce", ALU.add, replica_groups=[list(range(N_CORES))],
            ins=[cw_in.opt()], outs=[cw_out.opt()])

    bng = consts.tile([128, 1], F32, tag="bng", name="bng")
    bnb = consts.tile([128, 1], F32, tag="bnb", name="bnb")
    svin = consts.tile([128, 2 * BPC], F32, tag="svin", name="svin")
    nc.gpsimd.dma_start(out=bng, in_=io["bng"][:, :])
    nc.gpsimd.dma_start(out=bnb, in_=io["bnb"][:, :])
    nc.gpsimd.dma_start(out=svin, in_=io["svin"][:, :])

    # ---- PSUM: warm-up transpose in a throwaway pool ----
    with tc.tile_pool(name="ps_wm", bufs=1, space="PSUM") as pw:
        warm = pw.tile([128, 128], BF16, tag="warmt", name="warmt")
        nc.tensor.transpose(warm[:, :], ident[:, :], ident[:, :])

    # ---- resident SBUF tensors (issue queues chosen for criticality) ----
    # vT16: gpsimd queue, early -- needed by the V projection.
    vT16_sb = []
    for b in range(BPC):
        t16 = res.tile([128, 32, 128], BF16, tag=f"vT16_{b}", name=f"vT16_{b}")
        nc.gpsimd.dma_start(out=t16[:, :], in_=io["vT16"][b])
        vT16_sb.append(t16)

    # VW accumulators: outermost long-lived PSUM pool (2 banks)
    ps_vw = ctx.enter_context(tc.tile_pool(name="ps_vw", bufs=1, space="PSUM"))
    VWp = [ps_vw.tile([128, D], F32, tag=f"VWp{b}", name=f"VWp{b}") for b in range(BPC)]

    ps_qkv = tc.tile_pool(name="ps_qkv", bufs=1, space="PSUM")
    pa = ps_qkv.__enter__()
    QTp = [pa.tile([128, 4, 128], F32, tag=f"QTp{b}", name=f"QTp{b}") for b in range(BPC)]
    KTp = [pa.tile([128, 4, 128], F32, tag=f"KTp{b}", name=f"KTp{b}") for b in range(BPC)]
    Vp = [pa.tile([128, D], F32, tag=f"Vp{b}", name=f"Vp{b}") for b in range(BPC)]

    # ---- phase A1: Q^T/K^T projections (fp8 DR) + V (bf16), streaming ----
    # acts: 2 group-DMAs per (b, tensor) of 8 DR chunks each (256 KB)
    qgs, kgs = [], []
    for b in range(BPC):
        qg = [apool.tile([128, 8, 2, 128], FP8, tag=f"qg{b}{g}", name=f"qg{b}{g}")
              for g in range(2)]
        kg = [apool.tile([128, 8, 2, 128], FP8, tag=f"kg{b}{g}", name=f"kg{b}{g}")
              for g in range(2)]
        for g in range(2):
            nc.scalar.dma_start(out=qg[g], in_=io["qT8"][b, g])
            nc.scalar.dma_start(out=kg[g], in_=io["kT8"][b, g])
        qgs.append(qg); kgs.append(kg)
    # weights: 4 group-DMAs per tensor of 4 DR chunks each (512 KB)
    for g in range(4):
        wq_c = wpool.tile([128, 4, 2, D], FP8, tag="wq_c", name="wq_c")
        wk_c = wpool.tile([128, 4, 2, D], FP8, tag="wk_c", name="wk_c")
        nc.sync.dma_start(out=wq_c, in_=io["wq8"][g])
        nc.sync.dma_start(out=wk_c, in_=io["wk8"][g])
        wv_c = wpool.tile([128, 8, D], BF16, tag="wv_c", name="wv_c")
        nc.scalar.dma_start(out=wv_c, in_=io["wv16"][g])
        for sub in range(4):
            pc = 4 * g + sub
            for b in range(BPC):
                qc = qgs[b][pc // 8][:, pc % 8, :, :]
                kc = kgs[b][pc // 8][:, pc % 8, :, :]
                for db in range(4):
                    # one PSUM accumulation group per bank: start only on the
                    # first write into the bank, stop on the very last
                    st = pc == 0 and db == 0
                    sp = pc == NC4 - 1 and db == 3
                    nc.tensor.matmul(QTp[b][:, db, :],
                                     wq_c[:, sub, :, db * 128:(db + 1) * 128],
                                     qc, start=st, stop=sp, perf_mode=DR)
                    nc.tensor.matmul(KTp[b][:, db, :],
                                     wk_c[:, sub, :, db * 128:(db + 1) * 128],
                                     kc, start=st, stop=sp, perf_mode=DR)
            # V: two 128-row chunks per DR chunk (bf16, classic orientation)
            for half in range(2):
                pc2 = 2 * pc + half
                for b in range(BPC):
                    nc.tensor.matmul(Vp[b][:, :], vT16_sb[b][:, pc2, :],
                                     wv_c[:, pc2 % 8, :],
                                     start=pc2 == 0, stop=pc2 == 31)

    # marker: lands after the whole wq/wk stream on the sync queue; gates the
    # gpsimd residual stream below so it doesn't steal stage-1 bandwidth
    marker = consts.tile([128, 1], F32, tag="marker", name="marker")
    nc.sync.dma_start(out=marker, in_=io["bng"][:, :])

    # VW inputs behind the wq/wk stream on sync (needed ~mid-kernel)
    vT8_sb = []
    for b in range(BPC):
        t8 = res.tile([128, NC4, 2, 128], FP8, tag=f"vT8_{b}", name=f"vT8_{b}")
        nc.sync.dma_start(out=t8[:, :], in_=io["vT8"][b])
        vT8_sb.append(t8)
    wfn_sb = res.tile([128, NC4, 2, D], FP8, tag="wfn", name="wfn")
    for g in range(2):
        nc.sync.dma_start(out=wfn_sb[:, 8 * g:8 * g + 8], in_=io["wfn"][:, 8 * g:8 * g + 8])
    G_sb = res.tile([128, 4, D], BF16, tag="G", name="G")
    nc.scalar.dma_start(out=G_sb[:, :], in_=io["G"][:, :])
    wv2 = res.tile([128, 4, 2], BF16, tag="wv2", name="wv2")
    nc.scalar.dma_start(out=wv2[:, :], in_=io["wv2"][:, :])

    # ---- phase A2: VW = (v @ wfc_eff) * SC_VW via fp8 DR, accumulating ----
    for pc in range(NC4):
        for b in range(BPC):
            nc.tensor.matmul(VWp[b][:, :], vT8_sb[b][:, pc, :, :], wfn_sb[:, pc, :, :],
                             start=pc == 0, stop=pc == NC4 - 1, perf_mode=DR)

    # fc weights: scalar queue, behind the act/wv16 stream
    wfcT = res.tile([128, 4, P], BF16, tag="wfcT", name="wfcT")
    for dc in range(4):
        nc.scalar.dma_start(out=wfcT[:, dc], in_=io["wfcT"][dc])

    # ---- evacuate QT/KT/V to SBUF (bf16), then free those PSUM banks ----
    qkv_sb = []
    for b in range(BPC):
        QT_sb = sb.tile([128, 4, 128], BF16, tag="QT_sb", name="QT_sb")
        KT_sb = sb.tile([128, 4, 128], BF16, tag="KT_sb", name="KT_sb")
        V_sb = sb.tile([128, D], BF16, tag="V_sb", name="V_sb")
        nc.vector.tensor_copy(out=QT_sb, in_=QTp[b][:, :, :])
        nc.scalar.copy(out=KT_sb, in_=KTp[b][:, :, :])
        nc.vector.tensor_copy(out=V_sb, in_=Vp[b][:, :])
        qkv_sb.append((QT_sb, KT_sb, V_sb))
    ps_qkv.__exit__(None, None, None)

    # residual: gpsimd queue, gated behind the stage-1 marker
    mdump = dram.tile([128, 1], F32, tag="mdump", name="mdump")
    nc.gpsimd.dma_start(out=mdump[:, :], in_=marker)
    veff_sb = []
    for b in range(BPC):
        t = res.tile([128, P], BF16, tag=f"veff{b}", name=f"veff{b}")
        nc.gpsimd.dma_start(out=t[:, :], in_=io["veff"][b])
        veff_sb.append(t)

    # attention-era PSUM: one f32 bank (3 S slots + 2 a12 slots), one bf16
    # transpose bank (8 slots), one O bank, 2 fc banks (also used for Zp).
    # PSUM reserves a full 2KB bank per tag-buffer, so slots are hand-sliced.
    ps_at = ctx.enter_context(tc.tile_pool(name="ps_at", bufs=1, space="PSUM"))
    Sbank = ps_at.tile([128, 4, 128], F32, tag="Sbank", name="Sbank")
    Tbank = ps_at.tile([128, 8, 128], BF16, tag="Tbank", name="Tbank")
    Obank = [ps_at.tile([128, D], F32, tag=f"Ob{b}", name=f"Ob{b}") for b in range(BPC)]
    ps_fc = ctx.enter_context(tc.tile_pool(name="ps_fc", bufs=2, space="PSUM"))
    tslot = [0]

    def tslot_next():
        s = tslot[0] % 8
        tslot[0] += 1
        return s

    # ---- attention + silu + LN + stats per batch ----
    xTs = []
    cin_sb = small.tile([128, 2], F32, tag="cin_sb", name="cin_sb")
    s1l = [small.tile([128, 1], F32, tag=f"s1l{b}", name=f"s1l{b}") for b in range(BPC)]
    s2l = [small.tile([128, 1], F32, tag=f"s2l{b}", name=f"s2l{b}") for b in range(BPC)]
    for b in range(BPC):
        QT_sb, KT_sb, V_sb = qkv_sb[b]
        Opsum = Obank[b]
        Osc = sb.tile([128, D], F32, tag="Osc", name="Osc")
        for h in range(NH):
            po = (h % 2) * 64
            dc = h // 2
            S = Sbank[:, h % 3, :]
            nc.tensor.matmul(S, QT_sb[po:po + 64, dc, :], KT_sb[po:po + 64, dc, :],
                             start=True, stop=True)
            e_f = sb.tile([128, 128], BF16, tag="e_f", name="e_f")
            lsum = small.tile([128, 1], F32, tag="lsum", name="lsum")
            nc.scalar.activation(out=e_f, in_=S, func=AF.Exp, scale=esc,
                                 accum_out=lsum)
            rs = small.tile([128, 1], F32, tag="rs", name="rs")
            nc.vector.reciprocal(rs, lsum)
            tpa = Tbank[:, tslot_next(), :]
            nc.tensor.transpose(tpa, e_f[:, :], ident[:, :])
            aT = sb.tile([128, 128], BF16, tag="aT", name="aT")
            nc.scalar.copy(out=aT, in_=tpa)
            nc.tensor.matmul(Opsum[:, h * 64:(h + 1) * 64], aT[:, :],
                             V_sb[:, h * 64:(h + 1) * 64], start=True, stop=True)
            nc.vector.tensor_scalar_mul(out=Osc[:, h * 64:(h + 1) * 64],
                                        in0=Opsum[:, h * 64:(h + 1) * 64], scalar1=rs)

        # silu + layernorm (affine folded into wfc_eff/veff on host)
        sg = sb.tile([128, D], F32, tag="sg", name="sg")
        nc.scalar.activation(out=sg, in_=Osc, func=AF.Sigmoid)
        Osw = sb.tile([128, D], F32, tag="Osw", name="Osw")
        nc.vector.tensor_mul(out=Osw, in0=Osc, in1=sg)
        st6 = small.tile([128, 6], F32, tag="st6", name="st6")
        nc.vector.bn_stats(out=st6, in_=Osw)
        mv = small.tile([128, 2], F32, tag="mv", name="mv")
        nc.vector.bn_aggr(out=mv, in_=st6)
        sd = small.tile([128, 1], F32, tag="sd", name="sd")
        nc.scalar.activation(out=sd, in_=mv[:, 1:2], func=AF.Sqrt,
                             scale=float(D) / (D - 1))
        nc.vector.tensor_scalar_add(out=sd, in0=sd, scalar1=LN_EPS)
        rstd = small.tile([128, 1], F32, tag="rstd", name="rstd")
        nc.vector.reciprocal(rstd, sd)
        xhat = sb.tile([128, D], BF16, tag="xhat", name="xhat")
        nc.vector.tensor_scalar(out=xhat, in0=Osw, scalar1=mv[:, 0:1], scalar2=rstd,
                                op0=ALU.subtract, op1=ALU.mult)
        xT = sb.tile([128, 4, 128], BF16, tag="xT", name="xT")
        for dc in range(4):
            tp = Tbank[:, tslot_next(), :]
            nc.tensor.transpose(tp, xhat[:, dc * 128:(dc + 1) * 128], ident[:, :])
            nc.vector.tensor_copy(out=xT[:, dc, :], in_=tp)
        xTs.append(xT)

        # stats: S1 = sv1 + xhat.wsum ; S2 = sv2 + (2/SC)xhat.VW + 2 xhat.vwc + xhat.(xhat G)
        Zp = ps_fc.tile([128, D], F32, tag="O2", name="Zp")
        a12 = Sbank[:, 3, 2 * b:2 * b + 2]
        for dc in range(4):
            nc.tensor.matmul(Zp[:, :], xT[:, dc, :], G_sb[:, dc, :],
                             start=dc == 0, stop=dc == 3)
            nc.tensor.matmul(a12, xT[:, dc, :], wv2[:, dc, :],
                             start=dc == 0, stop=dc == 3)
        AX = mybir.AxisListType
        j1 = tpool.tile([128, D], BF16, tag="junk", name="junk1")
        j2 = tpool.tile([128, D], BF16, tag="junk", name="junk2")
        nc.vector.tensor_mul(out=j1, in0=xhat, in1=VWp[b][:, :])
        nc.vector.tensor_mul(out=j2, in0=xhat, in1=Zp[:, :])
        r1 = small.tile([128, 1], F32, tag="r1", name="r1")
        r2 = small.tile([128, 1], F32, tag="r2", name="r2")
        nc.vector.reduce_sum(r1, j1, axis=AX.X)
        nc.vector.reduce_sum(r2, j2, axis=AX.X)
        # S2 = sv2 + (2/SC_VW) r1 + r2 + a12[:,1]
        s2a = small.tile([128, 1], F32, tag="s2a", name="s2a")
        nc.vector.tensor_scalar(out=s2a, in0=r1, scalar1=2.0 / SC_VW,
                                scalar2=svin[:, 2 * b + 1:2 * b + 2],
                                op0=ALU.mult, op1=ALU.add)
        s2b = small.tile([128, 1], F32, tag="s2b", name="s2b")
        nc.vector.tensor_add(out=s2b, in0=s2a, in1=r2)
        nc.vector.tensor_add(out=s2l[b], in0=s2b, in1=a12[:, 1:2])
        nc.vector.tensor_add(out=s1l[b], in0=svin[:, 2 * b:2 * b + 1], in1=a12[:, 0:1])
    nc.vector.tensor_add(out=cin_sb[:, 0:1], in0=s1l[0], in1=s1l[1])
    nc.vector.tensor_add(out=cin_sb[:, 1:2], in0=s2l[0], in1=s2l[1])

    # ---- stats AllReduce (overlaps the fc phase below) ----
    cin = dram.tile([128, 2], F32, tag="cin", name="cin")
    cout = dram.tile([128, 2], F32, tag="cout", name="cout")
    nc.gpsimd.dma_start(out=cin[:, :], in_=cin_sb)
    if os.environ.get("V2_SKIP_COLL", "0") == "1":
        nc.gpsimd.dma_start(out=cout[:, :], in_=cin[:, :])
    else:
        nc.gpsimd.collective_compute(
            "AllReduce", ALU.add, replica_groups=[list(range(N_CORES))],
            ins=[cin.opt()], outs=[cout.opt()])
    red = small.tile([128, 2], F32, tag="red", name="red")
    nc.gpsimd.dma_start(out=red[:, :], in_=cout[:, :])

    # ---- fc matmuls + residual add (not AR-gated) ----
    tsegs = []
    for pt in range(8):
        for b in range(BPC):
            O2 = ps_fc.tile([128, 512], F32, tag="O2", name="O2")
            for dc in range(4):
                nc.tensor.matmul(O2[:, :], xTs[b][:, dc, :],
                                 wfcT[:, dc, pt * 512:(pt + 1) * 512],
                                 start=dc == 0, stop=dc == 3)
            tseg = tpool.tile([128, 512], BF16, tag="tseg", name="tseg")
            nc.vector.tensor_add(out=tseg, in0=veff_sb[b][:, pt * 512:(pt + 1) * 512],
                                 in1=O2[:, :])
            tsegs.append((b, pt, tseg))

    # ---- post-AR: BN affine factors ----
    inv_n = 1.0 / float(B * P)
    mean = small.tile([128, 1], F32, tag="mean", name="mean")
    nc.scalar.mul(out=mean, in_=red[:, 0:1], mul=inv_n)
    ex2 = small.tile([128, 1], F32, tag="ex2", name="ex2")
    nc.scalar.mul(out=ex2, in_=red[:, 1:2], mul=inv_n)
    msq = small.tile([128, 1], F32, tag="msq", name="msq")
    nc.vector.tensor_mul(out=msq, in0=mean, in1=mean)
    var = small.tile([128, 1], F32, tag="var", name="var")
    nc.vector.tensor_sub(out=var, in0=ex2, in1=msq)
    epsbn = consts.tile([128, 1], F32, tag="epsbn", name="epsbn")
    nc.vector.memset(epsbn, BN_EPS)
    sdv = small.tile([128, 1], F32, tag="sdv", name="sdv")
    nc.scalar.activation(out=sdv, in_=var, func=AF.Sqrt, bias=epsbn)
    invs = small.tile([128, 1], F32, tag="invs", name="invs")
    nc.vector.reciprocal(invs, sdv)
    scl = small.tile([128, 1], F32, tag="scl", name="scl")
    nc.vector.tensor_mul(out=scl, in0=bng, in1=invs)
    tmp = small.tile([128, 1], F32, tag="tmp", name="tmp")
    nc.vector.tensor_mul(out=tmp, in0=mean, in1=scl)
    shf = small.tile([128, 1], F32, tag="shf", name="shf")
    nc.vector.tensor_sub(out=shf, in0=bnb, in1=tmp)

    # ---- AR-gated epilogue: scale+shift, split ACT/DVE, store ----
    for i, (b, pt, tseg) in enumerate(tsegs):
        stage = stg.tile([128, 512], BF16, tag="stage", name="stage")
        if i % 2 == 0:
            nc.scalar.activation(out=stage, in_=tseg, func=AF.Identity,
                                 bias=shf, scale=scl)
        else:
            nc.vector.tensor_scalar(out=stage, in0=tseg, scalar1=scl, scalar2=shf,
                                    op0=ALU.mult, op1=ALU.add)
        eng = nc.sync if i % 2 == 0 else nc.gpsimd
        eng.dma_start(out=io["out"][b, :, pt * 512:(pt + 1) * 512], in_=stage)


def _build(temp):
    key = (MODE, temp, _ENV_KEY())
    if key in _BUILD_CACHE:
        return _BUILD_CACHE[key]
    nc = bacc.Bacc("TRN2", target_bir_lowering=False, debug=False, num_devices=N_CORES)
    io = {
        "qT8": nc.dram_tensor("qT8", [BPC, 2, 128, 8, 2, 128], FP8, kind="ExternalInput").ap(),
        "kT8": nc.dram_tensor("kT8", [BPC, 2, 128, 8, 2, 128], FP8, kind="ExternalInput").ap(),
        "vT8": nc.dram_tensor("vT8", [BPC, 128, NC4, 2, 128], FP8, kind="ExternalInput").ap(),
        "vT16": nc.dram_tensor("vT16", [BPC, 128, 32, 128], BF16, kind="ExternalInput").ap(),
        "veff": nc.dram_tensor("veff", [BPC, C, P], BF16, kind="ExternalInput").ap(),
        "wq8": nc.dram_tensor("wq8", [4, 128, 4, 2, D], FP8, kind="ExternalInput").ap(),
        "wk8": nc.dram_tensor("wk8", [4, 128, 4, 2, D], FP8, kind="ExternalInput").ap(),
        "wv16": nc.dram_tensor("wv16", [4, 128, 8, D], BF16, kind="ExternalInput").ap(),
        "wfn": nc.dram_tensor("wfn", [128, NC4, 2, D], FP8, kind="ExternalInput").ap(),
        "wfcT": nc.dram_tensor("wfcT", [4, 128, P], BF16, kind="ExternalInput").ap(),
        "G": nc.dram_tensor("G", [128, 4, D], BF16, kind="ExternalInput").ap(),
        "wv2": nc.dram_tensor("wv2", [128, 4, 2], BF16, kind="ExternalInput").ap(),
        "svin": nc.dram_tensor("svin", [C, 2 * BPC], F32, kind="ExternalInput").ap(),
        "bng": nc.dram_tensor("bng", [C, 1], F32, kind="ExternalInput").ap(),
        "bnb": nc.dram_tensor("bnb", [C, 1], F32, kind="ExternalInput").ap(),
        "out": nc.dram_tensor("out", [BPC, C, P], BF16, kind="ExternalOutput").ap(),
    }
    with tile.TileContext(nc) as tc, ExitStack() as ctx:
        _emit(ctx, nc, tc, io, temp)
    nc.compile()
    _BUILD_CACHE[key] = nc
    return nc


def _pack_dr_g(xT, ng, sub):
    # [P, M] -> [ng, 128, sub, 2, M]: group g, partition r, chunk sub, half i
    # covers row p = (g*sub + s)*256 + i*128 + r
    M = xT.shape[1]
    return np.ascontiguousarray(
        xT.reshape(ng, sub, 2, 128, M).transpose(0, 3, 1, 2, 4))


def _f8(x):
    return np.ascontiguousarray(np.asarray(x, np.float32).astype(ml_dtypes.float8_e4m3))


def _b16(x):
    return np.ascontiguousarray(np.asarray(x, np.float32).astype(ml_dtypes.bfloat16))


def kernel(v, k, q, w_qs, w_ks, w_vs, w_fc, ln_gamma, ln_beta, temperature,
           bn_gamma, bn_beta, **_ignored):
    v = np.asarray(v, np.float32)
    k = np.asarray(k, np.float32)
    q = np.asarray(q, np.float32)
    w_qs = np.asarray(w_qs, np.float32)
    w_ks = np.asarray(w_ks, np.float32)
    w_vs = np.asarray(w_vs, np.float32)
    w_fc = np.asarray(w_fc, np.float32)
    ln_gamma = np.asarray(ln_gamma, np.float32)
    ln_beta = np.asarray(ln_beta, np.float32)
    temp = float(np.asarray(temperature))
    bn_gamma = np.asarray(bn_gamma, np.float32)
    bn_beta = np.asarray(bn_beta, np.float32)

    qf = q.reshape(B, C, P)
    kf = k.reshape(B, C, P)
    vf = v.reshape(B, C, P)

    # activations: [P, C] transposed, group-DR-packed for q/k; r-major for vT8/vT16
    qT8 = np.stack([_f8(_pack_dr_g(qf[b].T, 2, 8)) for b in range(B)])
    kT8 = np.stack([_f8(_pack_dr_g(kf[b].T, 2, 8)) for b in range(B)])
    # vT8 SBUF layout [r, pc, i, c]:
    vT8 = np.stack([_f8(vf[b].T.reshape(NC4, 2, 128, C).transpose(2, 0, 1, 3))
                    for b in range(B)])
    # vT16 SBUF layout [r, pc2, c]:
    vT16 = np.stack([_b16(vf[b].T.reshape(32, 128, C).transpose(1, 0, 2))
                     for b in range(B)])

    # weights
    wq8 = _f8(_pack_dr_g(w_qs.T * SC_QK, 4, 4))
    wk8 = _f8(_pack_dr_g(w_ks.T * SC_QK, 4, 4))
    wv16 = _b16(w_vs.T.reshape(4, 8, 128, D).transpose(0, 2, 1, 3))
    wfc_eff = (w_fc * ln_gamma[None, :]).astype(np.float32)       # [P, D]
    # wfn SBUF layout [r, pc, i, d]:
    wfn8 = _f8((wfc_eff * SC_VW).reshape(NC4, 2, 128, D).transpose(2, 0, 1, 3))
    wfcT16 = _b16(wfc_eff.T.reshape(4, 128, P))
    G = (wfc_eff.T.astype(np.float64) @ wfc_eff.astype(np.float64)).astype(np.float32)
    G16 = _b16(G.reshape(4, 128, D).transpose(1, 0, 2))           # [r, dc, d]
    bias_fc = (w_fc @ ln_beta).astype(np.float32)                 # [P]
    wsum = wfc_eff.sum(0)
    vwc = (bias_fc.astype(np.float64) @ wfc_eff.astype(np.float64)).astype(np.float32)
    wv2 = _b16(np.stack([wsum, 2.0 * vwc], axis=1)
               .reshape(4, 128, 2).transpose(1, 0, 2))            # [r, dc, j]

    veff_f = vf + bias_fc[None, None, :]
    veff = _b16(veff_f)
    # per-batch input-residual partial sums for the BN stats decomposition
    sv1 = veff_f.astype(np.float64).sum(-1)                       # [B, C]
    sv2 = (veff_f.astype(np.float64) ** 2).sum(-1)
    svin_all = np.stack([sv1, sv2], axis=-1).astype(np.float32)   # [B, C, 2]

    bng = np.ascontiguousarray(bn_gamma.reshape(C, 1))
    bnb = np.ascontiguousarray(bn_beta.reshape(C, 1))

    nc = _build(temp)
    in_maps = []
    for i in range(N_CORES):
        bs = slice(BPC * i, BPC * (i + 1))
        # svin layout: [c, 2*b + (0:sum, 1:sumsq)]
        svin = np.ascontiguousarray(
            svin_all[bs].transpose(1, 0, 2).reshape(C, 2 * BPC))
        in_maps.append({
            "qT8": qT8[bs], "kT8": kT8[bs], "vT8": vT8[bs], "vT16": vT16[bs],
            "veff": veff[bs], "wq8": wq8, "wk8": wk8, "wv16": wv16,
            "wfn": wfn8, "wfcT": wfcT16, "G": G16, "wv2": wv2,
            "svin": svin, "bng": bng, "bnb": bnb,
        })
    res = run_bass_kernel_spmd(nc, in_maps, core_ids=list(range(N_CORES)))
    global LAST_RESULTS
    LAST_RESULTS = res
    out = np.concatenate([np.asarray(res.results[i]["out"], np.float32)
                          for i in range(N_CORES)], axis=0)
    return out.reshape(B, C, HH, WW)


# revision 39
# speedup vs baseline: 1.2619x; 1.0786x over previous
"""Trainium2 Bass kernel for nn_MultiHeadAttention (channel-attention block).

Math per batch (X* = reshape(*, [C,P]), P=4096, C=128, D=512, 8 heads x 64):
  Q^T = Wq^T Xq^T, K^T = Wk^T Xk^T   (computed directly transposed, fp8 DR)
  V   = Xv Wv^T                      (bf16)
  per head: e = exp(Q_h K_h^T * esc); O_h = (e / rowsum(e)) V_h
  O = silu(O); xhat = (O - mean)/(unbiased_std + eps)   (LN affine folded
  into wfc_eff = w_fc * ln_gamma and veff = v + w_fc @ ln_beta)
  out_pre = veff + xhat @ wfc_eff^T
  out = BatchNorm2d(out_pre), batch stats over (b,h,w)

BN statistics are computed BEFORE the fc matmul via the decomposition
  sum_p out   = sum_p veff + xhat . wsum
  sum_p out^2 = sum_p veff^2 + 2 xhat . (v @ wfc_eff + vwc) + xhat . (xhat G)
with G = wfc_eff^T wfc_eff, wsum = sum_p wfc_eff, vwc = bias_fc @ wfc_eff
precomputed on host and VW = v @ wfc_eff accumulated on device during the
DMA-bound load phase.  The 1KB AllReduce of the stats therefore overlaps the
fc matmul instead of being a serial ~38us tail (a warm-up AllReduce at kernel
start absorbs the CC entry barrier, cutting the real AR to ~12-23us), and the
BN affine is fused into the fc epilogue.

Scheduling notes (engine FIFOs are in-order):
 - the scalar (ACT) queue only issues the early activation/wv DMAs; mid-kernel
   bulk loads go on gpsimd/sync so they can't stall ACT compute.
 - the ACT head loop is exp-only (aT copies on DVE) -- mixing activation
   functions thrashes the ACT function tables.
 - S/exp/transpose head work is interleaved with the V-projection matmul
   stream so attention latency hides under the wv DMA.
 - all output stores go on the sync (HWDGE) queue; gpsimd SWDGE costs ~2us
   fixed per DMA.

Sharding: data-parallel over batch, 2 batches per core on 8 cores; BN stats
combined with a [128,2] AllReduce.

Dtypes: fp8e4 (x256 pre-scale, folded into the exp scale) for the Q/K path
and (x32) for the VW stats matmul -- both DoubleRow at 2x PE rate; bf16 for
the V / fc / residual path (the LN amplifies O errors ~20x there, fp8 fails
the 2e-2 gate; verified numerically on host).
"""

import os
from contextlib import ExitStack

import ml_dtypes
import numpy as np

import concourse.mybir as mybir
import concourse.tile as tile
from concourse import bacc
from concourse.bass_utils import run_bass_kernel_spmd
from concourse.masks import make_identity

# ---- problem constants (hardcoded per contract) ----
B, C, HH, WW = 16, 128, 64, 64
P = HH * WW           # 4096
NH, LD = 8, 64
D = NH * LD           # 512
N_CORES = 8
BPC = B // N_CORES    # 2 batches per core
NC4 = 16              # 256-row contraction chunks (DoubleRow)
LN_EPS = 1e-6
BN_EPS = 1e-5
F32 = mybir.dt.float32
BF16 = mybir.dt.bfloat16
FP8 = mybir.dt.float8e4
DR = mybir.MatmulPerfMode.DoubleRow

SC_QK = 256.0         # fp8 pre-scale for wq/wk (keeps them out of subnormals)
SC_VW = 32.0          # fp8 pre-scale for wfn in the VW stats matmul

MODE = "v3"           # printed by test.py
_ENV_KEY = lambda: (os.environ.get("V2_WARMAR", "1"), os.environ.get("V2_SKIP_COLL", "0"))
_BUILD_CACHE: dict = {}
LAST_RESULTS = None


def _emit(ctx, nc, tc, io, temp):
    AF = mybir.ActivationFunctionType
    ALU = mybir.AluOpType
    AX = mybir.AxisListType
    esc = 1.0 / (SC_QK * SC_QK * temp)   # exp arg: undo fp8 pre-scale + temperature

    consts = ctx.enter_context(tc.tile_pool(name="consts", bufs=1))
    wpool = ctx.enter_context(tc.tile_pool(name="wpool", bufs=2))
    apool = ctx.enter_context(tc.tile_pool(name="apool", bufs=1))
    res = ctx.enter_context(tc.tile_pool(name="res", bufs=1))    # resident
    sb = ctx.enter_context(tc.tile_pool(name="sb", bufs=2))
    att = ctx.enter_context(tc.tile_pool(name="att", bufs=1))
    small = ctx.enter_context(tc.tile_pool(name="small", bufs=6))
    stg = ctx.enter_context(tc.tile_pool(name="stg", bufs=4))
    tpool = ctx.enter_context(tc.tile_pool(name="tp", bufs=4))
    dram = ctx.enter_context(tc.tile_pool(name="dram", bufs=1, space="DRAM"))

    # identity for PE transposes (bf16)
    ident_f = consts.tile([128, 128], F32, tag="identf", name="identf")
    make_identity(nc, ident_f)
    ident = consts.tile([128, 128], BF16, tag="ident", name="ident")
    nc.vector.tensor_copy(out=ident, in_=ident_f)

    # ---- warm-up collective: absorbs the CC entry barrier early ----
    if os.environ.get("V2_WARMAR", "1") == "1":
        cw_in = dram.tile([128, 1], F32, tag="cw_in", name="cw_in")
        cw_out = dram.tile([128, 1], F32, tag="cw_out", name="cw_out")
        warm_sb = consts.tile([128, 1], F32, tag="warm_sb", name="warm_sb")
        nc.vector.memset(warm_sb, 0.0)
        nc.gpsimd.dma_start(out=cw_in[:, :], in_=warm_sb)
        nc.gpsimd.collective_compute(
            "AllReduce", ALU.add, replica_groups=[list(range(N_CORES))],
            ins=[cw_in.opt()], outs=[cw_out.opt()])

    bng = consts.tile([128, 1], F32, tag="bng", name="bng")
    bnb = consts.tile([128, 1], F32, tag="bnb", name="bnb")
    svin = consts.tile([128, 2 * BPC], F32, tag="svin", name="svin")
    nc.gpsimd.dma_start(out=bng, in_=io["bng"][:, :])
    nc.gpsimd.dma_start(out=bnb, in_=io["bnb"][:, :])
    nc.gpsimd.dma_start(out=svin, in_=io["svin"][:, :])

    # ---- PSUM: warm-up transpose in a throwaway pool ----
    with tc.tile_pool(name="ps_wm", bufs=1, space="PSUM") as pw:
        warm = pw.tile([128, 128], BF16, tag="warmt", name="warmt")
        nc.tensor.transpose(warm[:, :], ident[:, :], ident[:, :])

    # vT16: gpsimd queue, first -- needed by the V projection.
    vT16_sb = []
    for b in range(BPC):
        t16 = res.tile([128, 32, 128], BF16, tag=f"vT16_{b}", name=f"vT16_{b}")
        nc.gpsimd.dma_start(out=t16[:, :], in_=io["vT16"][b])
        vT16_sb.append(t16)

    # attention-era PSUM: one f32 bank (3 S slots + 2 a12 slots) and one bf16
    # transpose bank (8 slots); lives for the whole kernel (2 banks).
    # PSUM reserves a full 2KB bank per tag, so slots are hand-sliced.
    ps_at = ctx.enter_context(tc.tile_pool(name="ps_at", bufs=1, space="PSUM"))
    Sbank = ps_at.tile([128, 4, 128], F32, tag="Sbank", name="Sbank")
    Tbank = ps_at.tile([128, 8, 128], BF16, tag="Tbank", name="Tbank")
    tslot = [0]

    def tslot_next():
        s = tslot[0] % 8
        tslot[0] += 1
        return s

    ps_qkv = tc.tile_pool(name="ps_qkv", bufs=1, space="PSUM")
    pa = ps_qkv.__enter__()
    QTp = [pa.tile([128, 4, 128], F32, tag=f"QTp{b}", name=f"QTp{b}") for b in range(BPC)]
    KTp = [pa.tile([128, 4, 128], F32, tag=f"KTp{b}", name=f"KTp{b}") for b in range(BPC)]
    Vp = [pa.tile([128, D], F32, tag=f"Vp{b}", name=f"Vp{b}") for b in range(BPC)]

    # ---- phase A-QK: Q^T/K^T projections (fp8 DR), streaming ----
    # acts: 2 group-DMAs per (b, tensor) of 8 DR chunks each (256 KB), scalar q
    qgs, kgs = [], []
    for b in range(BPC):
        qg = [apool.tile([128, 8, 2, 128], FP8, tag=f"qg{b}{g}", name=f"qg{b}{g}")
              for g in range(2)]
        kg = [apool.tile([128, 8, 2, 128], FP8, tag=f"kg{b}{g}", name=f"kg{b}{g}")
              for g in range(2)]
        for g in range(2):
            nc.scalar.dma_start(out=qg[g], in_=io["qT8"][b, g])
            nc.scalar.dma_start(out=kg[g], in_=io["kT8"][b, g])
        qgs.append(qg); kgs.append(kg)
    # weights: 4 group-DMAs per tensor of 4 DR chunks each (512 KB), sync q
    for g in range(4):
        wq_c = wpool.tile([128, 4, 2, D], FP8, tag="wq_c", name="wq_c")
        wk_c = wpool.tile([128, 4, 2, D], FP8, tag="wk_c", name="wk_c")
        nc.sync.dma_start(out=wq_c, in_=io["wq8"][g])
        nc.sync.dma_start(out=wk_c, in_=io["wk8"][g])
        for sub in range(4):
            pc = 4 * g + sub
            for b in range(BPC):
                qc = qgs[b][pc // 8][:, pc % 8, :, :]
                kc = kgs[b][pc // 8][:, pc % 8, :, :]
                for db in range(4):
                    # one PSUM accumulation group per bank: start only on the
                    # first write into the bank, stop on the very last
                    st = pc == 0 and db == 0
                    sp = pc == NC4 - 1 and db == 3
                    nc.tensor.matmul(QTp[b][:, db, :],
                                     wq_c[:, sub, :, db * 128:(db + 1) * 128],
                                     qc, start=st, stop=sp, perf_mode=DR)
                    nc.tensor.matmul(KTp[b][:, db, :],
                                     wk_c[:, sub, :, db * 128:(db + 1) * 128],
                                     kc, start=st, stop=sp, perf_mode=DR)

    # marker: lands after the whole wq/wk stream on the sync queue; gates the
    # gpsimd bulk loads below so they don't steal stage-1 bandwidth
    marker = consts.tile([128, 1], F32, tag="marker", name="marker")
    nc.sync.dma_start(out=marker, in_=io["bng"][:, :])

    # VW inputs behind the wq/wk stream on sync (needed ~mid-kernel)
    vT8_sb = []
    for b in range(BPC):
        t8 = res.tile([128, NC4, 2, 128], FP8, tag=f"vT8_{b}", name=f"vT8_{b}")
        nc.sync.dma_start(out=t8[:, :], in_=io["vT8"][b])
        vT8_sb.append(t8)
    wfn_sb = res.tile([128, NC4, 2, D], FP8, tag="wfn", name="wfn")
    for g in range(2):
        nc.sync.dma_start(out=wfn_sb[:, 8 * g:8 * g + 8], in_=io["wfn"][:, 8 * g:8 * g + 8])
    G_sb = res.tile([128, 4, D], BF16, tag="G", name="G")
    nc.scalar.dma_start(out=G_sb[:, :], in_=io["G"][:, :])
    wv2 = res.tile([128, 4, 2], BF16, tag="wv2", name="wv2")
    nc.scalar.dma_start(out=wv2[:, :], in_=io["wv2"][:, :])

    # residual + fc weights: gpsimd queue, gated behind the stage-1 marker
    mdump = dram.tile([128, 1], F32, tag="mdump", name="mdump")
    nc.gpsimd.dma_start(out=mdump[:, :], in_=marker)
    veff_sb = []
    for b in range(BPC):
        t = res.tile([128, P], BF16, tag=f"veff{b}", name=f"veff{b}")
        nc.gpsimd.dma_start(out=t[:, :], in_=io["veff"][b])
        veff_sb.append(t)
    wfcT = res.tile([128, 4, P], BF16, tag="wfcT", name="wfcT")
    for dc in range(4):
        nc.gpsimd.dma_start(out=wfcT[:, dc], in_=io["wfcT"][dc])

    # ---- evacuate QT/KT to SBUF (bf16) ----
    qkv_sb = []
    for b in range(BPC):
        QT_sb = sb.tile([128, 4, 128], BF16, tag="QT_sb", name="QT_sb")
        KT_sb = sb.tile([128, 4, 128], BF16, tag="KT_sb", name="KT_sb")
        nc.vector.tensor_copy(out=QT_sb, in_=QTp[b][:, :, :])
        nc.scalar.copy(out=KT_sb, in_=KTp[b][:, :, :])
        qkv_sb.append([QT_sb, KT_sb, None])

    # ---- phase A-V (bf16 stream) interleaved with S/exp head work ----
    # heads: (b, h) pairs; emit 4 per wv group BEFORE that group's V matmuls
    heads = [(b, h) for h in range(NH) for b in range(BPC)]
    aTs = {}
    rss = {}
    for g in range(4):
        for (b, h) in heads[4 * g:4 * g + 4]:
            QT_sb, KT_sb, _ = qkv_sb[b]
            po = (h % 2) * 64
            dc = h // 2
            S = Sbank[:, (2 * b + h) % 3, :]
            nc.tensor.matmul(S, QT_sb[po:po + 64, dc, :], KT_sb[po:po + 64, dc, :],
                             start=True, stop=True)
            e_f = sb.tile([128, 128], BF16, tag="e_f", name="e_f")
            lsum = small.tile([128, 1], F32, tag="lsum", name="lsum")
            nc.scalar.activation(out=e_f, in_=S, func=AF.Exp, scale=esc,
                                 accum_out=lsum)
            rs = small.tile([128, 1], F32, tag="rs", name="rs")
            nc.vector.reciprocal(rs, lsum)
            tpa = Tbank[:, tslot_next(), :]
            nc.tensor.transpose(tpa, e_f[:, :], ident[:, :])
            aT = att.tile([128, 128], BF16, tag=f"aT{b}{h}", name=f"aT{b}{h}")
            nc.vector.tensor_copy(out=aT, in_=tpa)
            aTs[(b, h)] = aT
            rss[(b, h)] = rs
        wv_c = wpool.tile([128, 8, D], BF16, tag="wv_c", name="wv_c")
        nc.scalar.dma_start(out=wv_c, in_=io["wv16"][g])
        for sub in range(8):
            pc2 = 8 * g + sub
            for b in range(BPC):
                nc.tensor.matmul(Vp[b][:, :], vT16_sb[b][:, pc2, :],
                                 wv_c[:, sub, :],
                                 start=pc2 == 0, stop=pc2 == 31)

    # evacuate V, free QKV PSUM banks
    for b in range(BPC):
        V_sb = sb.tile([128, D], BF16, tag="V_sb", name="V_sb")
        nc.vector.tensor_copy(out=V_sb, in_=Vp[b][:, :])
        qkv_sb[b][2] = V_sb
    ps_qkv.__exit__(None, None, None)

    # post-phase-A PSUM pools (2 banks each): VW accumulators, O banks, fc
    ps_vw = ctx.enter_context(tc.tile_pool(name="ps_vw", bufs=1, space="PSUM"))
    VWp = [ps_vw.tile([128, D], F32, tag=f"VWp{b}", name=f"VWp{b}") for b in range(BPC)]
    ps_ao = ctx.enter_context(tc.tile_pool(name="ps_ao", bufs=1, space="PSUM"))
    Obank = [ps_ao.tile([128, D], F32, tag=f"Ob{b}", name=f"Ob{b}") for b in range(BPC)]
    ps_fc = ctx.enter_context(tc.tile_pool(name="ps_fc", bufs=2, space="PSUM"))

    # ---- A@V for all heads ----
    Oscs = []
    for b in range(BPC):
        Osc = sb.tile([128, D], F32, tag="Osc", name="Osc")
        Oscs.append(Osc)
    for (b, h) in heads:
        nc.tensor.matmul(Obank[b][:, h * 64:(h + 1) * 64], aTs[(b, h)][:, :],
                         qkv_sb[b][2][:, h * 64:(h + 1) * 64], start=True, stop=True)
        nc.vector.tensor_scalar_mul(out=Oscs[b][:, h * 64:(h + 1) * 64],
                                    in0=Obank[b][:, h * 64:(h + 1) * 64],
                                    scalar1=rss[(b, h)])

    # ---- VW = (v @ wfc_eff) * SC_VW via fp8 DR, accumulating ----
    for pc in range(NC4):
        for b in range(BPC):
            nc.tensor.matmul(VWp[b][:, :], vT8_sb[b][:, pc, :, :], wfn_sb[:, pc, :, :],
                             start=pc == 0, stop=pc == NC4 - 1, perf_mode=DR)

    # ---- silu + LN + stats per batch ----
    xTs = []
    cin_sb = small.tile([128, 2], F32, tag="cin_sb", name="cin_sb")
    s1l = [small.tile([128, 1], F32, tag=f"s1l{b}", name=f"s1l{b}") for b in range(BPC)]
    s2l = [small.tile([128, 1], F32, tag=f"s2l{b}", name=f"s2l{b}") for b in range(BPC)]
    for b in range(BPC):
        Osc = Oscs[b]
        sg = sb.tile([128, D], F32, tag="sg", name="sg")
        nc.scalar.activation(out=sg, in_=Osc, func=AF.Sigmoid)
        Osw = sb.tile([128, D], F32, tag="Osw", name="Osw")
        nc.vector.tensor_mul(out=Osw, in0=Osc, in1=sg)
        st6 = small.tile([128, 6], F32, tag="st6", name="st6")
        nc.vector.bn_stats(out=st6, in_=Osw)
        mv = small.tile([128, 2], F32, tag="mv", name="mv")
        nc.vector.bn_aggr(out=mv, in_=st6)
        sd = small.tile([128, 1], F32, tag="sd", name="sd")
        nc.scalar.activation(out=sd, in_=mv[:, 1:2], func=AF.Sqrt,
                             scale=float(D) / (D - 1))
        nc.vector.tensor_scalar_add(out=sd, in0=sd, scalar1=LN_EPS)
        rstd = small.tile([128, 1], F32, tag="rstd", name="rstd")
        nc.vector.reciprocal(rstd, sd)
        xhat = sb.tile([128, D], BF16, tag="xhat", name="xhat")
        nc.vector.tensor_scalar(out=xhat, in0=Osw, scalar1=mv[:, 0:1], scalar2=rstd,
                                op0=ALU.subtract, op1=ALU.mult)
        xT = sb.tile([128, 4, 128], BF16, tag="xT", name="xT")
        for dc in range(4):
            tp = Tbank[:, tslot_next(), :]
            nc.tensor.transpose(tp, xhat[:, dc * 128:(dc + 1) * 128], ident[:, :])
            nc.vector.tensor_copy(out=xT[:, dc, :], in_=tp)
        xTs.append(xT)

        # stats: S1 = sv1 + xhat.wsum ; S2 = sv2 + (2/SC)xhat.VW + 2 xhat.vwc + xhat.(xhat G)
        Zp = ps_fc.tile([128, D], F32, tag="O2", name="Zp")
        a12 = Sbank[:, 3, 2 * b:2 * b + 2]
        for dc in range(4):
            nc.tensor.matmul(Zp[:, :], xT[:, dc, :], G_sb[:, dc, :],
                             start=dc == 0, stop=dc == 3)
            nc.tensor.matmul(a12, xT[:, dc, :], wv2[:, dc, :],
                             start=dc == 0, stop=dc == 3)
        j1 = tpool.tile([128, D], BF16, tag="junk", name="junk1")
        j2 = tpool.tile([128, D], BF16, tag="junk", name="junk2")
        nc.vector.tensor_mul(out=j1, in0=xhat, in1=VWp[b][:, :])
        nc.vector.tensor_mul(out=j2, in0=xhat, in1=Zp[:, :])
        r1 = small.tile([128, 1], F32, tag="r1", name="r1")
        r2 = small.tile([128, 1], F32, tag="r2", name="r2")
        nc.vector.reduce_sum(r1, j1, axis=AX.X)
        nc.vector.reduce_sum(r2, j2, axis=AX.X)
        s2a = small.tile([128, 1], F32, tag="s2a", name="s2a")
        nc.vector.tensor_scalar(out=s2a, in0=r1, scalar1=2.0 / SC_VW,
                                scalar2=svin[:, 2 * b + 1:2 * b + 2],
                                op0=ALU.mult, op1=ALU.add)
        s2b = small.tile([128, 1], F32, tag="s2b", name="s2b")
        nc.vector.tensor_add(out=s2b, in0=s2a, in1=r2)
        nc.vector.tensor_add(out=s2l[b], in0=s2b, in1=a12[:, 1:2])
        nc.vector.tensor_add(out=s1l[b], in0=svin[:, 2 * b:2 * b + 1], in1=a12[:, 0:1])
    nc.vector.tensor_add(out=cin_sb[:, 0:1], in0=s1l[0], in1=s1l[1])
    nc.vector.tensor_add(out=cin_sb[:, 1:2], in0=s2l[0], in1=s2l[1])

    # ---- stats AllReduce (overlaps the fc phase below) ----
    cin = dram.tile([128, 2], F32, tag="cin", name="cin")
    cout = dram.tile([128, 2], F32, tag="cout", name="cout")
    nc.gpsimd.dma_start(out=cin[:, :], in_=cin_sb)
    if os.environ.get("V2_SKIP_COLL", "0") == "1":
        nc.gpsimd.dma_start(out=cout[:, :], in_=cin[:, :])
    else:
        nc.gpsimd.collective_compute(
            "AllReduce", ALU.add, replica_groups=[list(range(N_CORES))],
            ins=[cin.opt()], outs=[cout.opt()])
    red = small.tile([128, 2], F32, tag="red", name="red")
    nc.gpsimd.dma_start(out=red[:, :], in_=cout[:, :])

    # ---- fc matmuls + residual add (not AR-gated) ----
    tsegs = []
    for pt in range(8):
        for b in range(BPC):
            O2 = ps_fc.tile([128, 512], F32, tag="O2", name="O2")
            for dc in range(4):
                nc.tensor.matmul(O2[:, :], xTs[b][:, dc, :],
                                 wfcT[:, dc, pt * 512:(pt + 1) * 512],
                                 start=dc == 0, stop=dc == 3)
            tseg = tpool.tile([128, 512], BF16, tag="tseg", name="tseg")
            nc.vector.tensor_add(out=tseg, in0=veff_sb[b][:, pt * 512:(pt + 1) * 512],
                                 in1=O2[:, :])
            tsegs.append((b, pt, tseg))

    # ---- post-AR: BN affine factors ----
    inv_n = 1.0 / float(B * P)
    mean = small.tile([128, 1], F32, tag="mean", name="mean")
    nc.scalar.mul(out=mean, in_=red[:, 0:1], mul=inv_n)
    ex2 = small.tile([128, 1], F32, tag="ex2", name="ex2")
    nc.scalar.mul(out=ex2, in_=red[:, 1:2], mul=inv_n)
    msq = small.tile([128, 1], F32, tag="msq", name="msq")
    nc.vector.tensor_mul(out=msq, in0=mean, in1=mean)
    var = small.tile([128, 1], F32, tag="var", name="var")
    nc.vector.tensor_sub(out=var, in0=ex2, in1=msq)
    epsbn = consts.tile([128, 1], F32, tag="epsbn", name="epsbn")
    nc.vector.memset(epsbn, BN_EPS)
    sdv = small.tile([128, 1], F32, tag="sdv", name="sdv")
    nc.scalar.activation(out=sdv, in_=var, func=AF.Sqrt, bias=epsbn)
    invs = small.tile([128, 1], F32, tag="invs", name="invs")
    nc.vector.reciprocal(invs, sdv)
    scl = small.tile([128, 1], F32, tag="scl", name="scl")
    nc.vector.tensor_mul(out=scl, in0=bng, in1=invs)
    tmp = small.tile([128, 1], F32, tag="tmp", name="tmp")
    nc.vector.tensor_mul(out=tmp, in0=mean, in1=scl)
    shf = small.tile([128, 1], F32, tag="shf", name="shf")
    nc.vector.tensor_sub(out=shf, in0=bnb, in1=tmp)

    # ---- AR-gated epilogue: scale+shift, split ACT/DVE, store on sync ----
    for i, (b, pt, tseg) in enumerate(tsegs):
        stage = stg.tile([128, 512], BF16, tag="stage", name="stage")
        if i % 2 == 0:
            nc.scalar.activation(out=stage, in_=tseg, func=AF.Identity,
                                 bias=shf, scale=scl)
        else:
            nc.vector.tensor_scalar(out=stage, in0=tseg, scalar1=scl, scalar2=shf,
                                    op0=ALU.mult, op1=ALU.add)
        nc.sync.dma_start(out=io["out"][b, :, pt * 512:(pt + 1) * 512], in_=stage)


def _build(temp):
    key = (MODE, temp, _ENV_KEY())
    if key in _BUILD_CACHE:
        return _BUILD_CACHE[key]
    nc = bacc.Bacc("TRN2", target_bir_lowering=False, debug=False, num_devices=N_CORES)
    io = {
        "qT8": nc.dram_tensor("qT8", [BPC, 2, 128, 8, 2, 128], FP8, kind="ExternalInput").ap(),
        "kT8": nc.dram_tensor("kT8", [BPC, 2, 128, 8, 2, 128], FP8, kind="ExternalInput").ap(),
        "vT8": nc.dram_tensor("vT8", [BPC, 128, NC4, 2, 128], FP8, kind="ExternalInput").ap(),
        "vT16": nc.dram_tensor("vT16", [BPC, 128, 32, 128], BF16, kind="ExternalInput").ap(),
        "veff": nc.dram_tensor("veff", [BPC, C, P], BF16, kind="ExternalInput").ap(),
        "wq8": nc.dram_tensor("wq8", [4, 128, 4, 2, D], FP8, kind="ExternalInput").ap(),
        "wk8": nc.dram_tensor("wk8", [4, 128, 4, 2, D], FP8, kind="ExternalInput").ap(),
        "wv16": nc.dram_tensor("wv16", [4, 128, 8, D], BF16, kind="ExternalInput").ap(),
        "wfn": nc.dram_tensor("wfn", [128, NC4, 2, D], FP8, kind="ExternalInput").ap(),
        "wfcT": nc.dram_tensor("wfcT", [4, 128, P], BF16, kind="ExternalInput").ap(),
        "G": nc.dram_tensor("G", [128, 4, D], BF16, kind="ExternalInput").ap(),
        "wv2": nc.dram_tensor("wv2", [128, 4, 2], BF16, kind="ExternalInput").ap(),
        "svin": nc.dram_tensor("svin", [C, 2 * BPC], F32, kind="ExternalInput").ap(),
        "bng": nc.dram_tensor("bng", [C, 1], F32, kind="ExternalInput").ap(),
        "bnb": nc.dram_tensor("bnb", [C, 1], F32, kind="ExternalInput").ap(),
        "out": nc.dram_tensor("out", [BPC, C, P], BF16, kind="ExternalOutput").ap(),
    }
    with tile.TileContext(nc) as tc, ExitStack() as ctx:
        _emit(ctx, nc, tc, io, temp)
    nc.compile()
    _BUILD_CACHE[key] = nc
    return nc


def _pack_dr_g(xT, ng, sub):
    # [P, M] -> [ng, 128, sub, 2, M]: group g, partition r, chunk sub, half i
    # covers row p = (g*sub + s)*256 + i*128 + r
    M = xT.shape[1]
    return np.ascontiguousarray(
        xT.reshape(ng, sub, 2, 128, M).transpose(0, 3, 1, 2, 4))


def _f8(x):
    return np.ascontiguousarray(np.asarray(x, np.float32).astype(ml_dtypes.float8_e4m3))


def _b16(x):
    return np.ascontiguousarray(np.asarray(x, np.float32).astype(ml_dtypes.bfloat16))


def kernel(v, k, q, w_qs, w_ks, w_vs, w_fc, ln_gamma, ln_beta, temperature,
           bn_gamma, bn_beta, **_ignored):
    v = np.asarray(v, np.float32)
    k = np.asarray(k, np.float32)
    q = np.asarray(q, np.float32)
    w_qs = np.asarray(w_qs, np.float32)
    w_ks = np.asarray(w_ks, np.float32)
    w_vs = np.asarray(w_vs, np.float32)
    w_fc = np.asarray(w_fc, np.float32)
    ln_gamma = np.asarray(ln_gamma, np.float32)
    ln_beta = np.asarray(ln_beta, np.float32)
    temp = float(np.asarray(temperature))
    bn_gamma = np.asarray(bn_gamma, np.float32)
    bn_beta = np.asarray(bn_beta, np.float32)

    qf = q.reshape(B, C, P)
    kf = k.reshape(B, C, P)
    vf = v.reshape(B, C, P)

    # activations: [P, C] transposed, group-DR-packed for q/k; r-major for vT8/vT16
    qT8 = np.stack([_f8(_pack_dr_g(qf[b].T, 2, 8)) for b in range(B)])
    kT8 = np.stack([_f8(_pack_dr_g(kf[b].T, 2, 8)) for b in range(B)])
    # vT8 SBUF layout [r, pc, i, c]:
    vT8 = np.stack([_f8(vf[b].T.reshape(NC4, 2, 128, C).transpose(2, 0, 1, 3))
                    for b in range(B)])
    # vT16 SBUF layout [r, pc2, c]:
    vT16 = np.stack([_b16(vf[b].T.reshape(32, 128, C).transpose(1, 0, 2))
                     for b in range(B)])

    # weights
    wq8 = _f8(_pack_dr_g(w_qs.T * SC_QK, 4, 4))
    wk8 = _f8(_pack_dr_g(w_ks.T * SC_QK, 4, 4))
    wv16 = _b16(w_vs.T.reshape(4, 8, 128, D).transpose(0, 2, 1, 3))
    wfc_eff = (w_fc * ln_gamma[None, :]).astype(np.float32)       # [P, D]
    # wfn SBUF layout [r, pc, i, d]:
    wfn8 = _f8((wfc_eff * SC_VW).reshape(NC4, 2, 128, D).transpose(2, 0, 1, 3))
    wfcT16 = _b16(wfc_eff.T.reshape(4, 128, P))
    G = (wfc_eff.T.astype(np.float64) @ wfc_eff.astype(np.float64)).astype(np.float32)
    G16 = _b16(G.reshape(4, 128, D).transpose(1, 0, 2))           # [r, dc, d]
    bias_fc = (w_fc @ ln_beta).astype(np.float32)                 # [P]
    wsum = wfc_eff.sum(0)
    vwc = (bias_fc.astype(np.float64) @ wfc_eff.astype(np.float64)).astype(np.float32)
    wv2 = _b16(np.stack([wsum, 2.0 * vwc], axis=1)
               .reshape(4, 128, 2).transpose(1, 0, 2))            # [r, dc, j]

    veff_f = vf + bias_fc[None, None, :]
    veff = _b16(veff_f)
    # per-batch input-residual partial sums for the BN stats decomposition
    sv1 = veff_f.astype(np.float64).sum(-1)                       # [B, C]
    sv2 = (veff_f.astype(np.float64) ** 2).sum(-1)
    svin_all = np.stack([sv1, sv2], axis=-1).astype(np.float32)   # [B, C, 2]

    bng = np.ascontiguousarray(bn_gamma.reshape(C, 1))
    bnb = np.ascontiguousarray(bn_beta.reshape(C, 1))

    nc = _build(temp)
    in_maps = []
    for i in range(N_CORES):
        bs = slice(BPC * i, BPC * (i + 1))
        # svin layout: [c, 2*b + (0:sum, 1:sumsq)]
        svin = np.ascontiguousarray(
            svin_all[bs].transpose(1, 0, 2).reshape(C, 2 * BPC))
        in_maps.append({
            "qT8": qT8[bs], "kT8": kT8[bs], "vT8": vT8[bs], "vT16": vT16[bs],
            "veff": veff[bs], "wq8": wq8, "wk8": wk8, "wv16": wv16,
            "wfn": wfn8, "wfcT": wfcT16, "G": G16, "wv2": wv2,
            "svin": svin, "bng": bng, "bnb": bnb,
        })
    res = run_bass_kernel_spmd(nc, in_maps, core_ids=list(range(N_CORES)))
    global LAST_RESULTS
    LAST_RESULTS = res
    out = np.concatenate([np.asarray(res.results[i]["out"], np.float32)
                          for i in range(N_CORES)], axis=0)
    return out.reshape(B, C, HH, WW)


# revision 44
# speedup vs baseline: 1.2642x; 1.0018x over previous
"""Trainium2 Bass kernel for nn_MultiHeadAttention (channel-attention block).

Math per batch (X* = reshape(*, [C,P]), P=4096, C=128, D=512, 8 heads x 64):
  Q^T = Wq^T Xq^T, K^T = Wk^T Xk^T   (computed directly transposed, fp8 DR)
  V   = Xv Wv^T                      (bf16)
  per head: e = exp(Q_h K_h^T * esc); O_h = (e / rowsum(e)) V_h
  O = silu(O); xhat = (O - mean)/(unbiased_std + eps)   (LN affine folded
  into wfc_eff = w_fc * ln_gamma and veff = v + w_fc @ ln_beta)
  out_pre = veff + xhat @ wfc_eff^T
  out = BatchNorm2d(out_pre), batch stats over (b,h,w)

BN statistics are computed BEFORE the fc matmul via the decomposition
  sum_p out   = sum_p veff + xhat . wsum
  sum_p out^2 = sum_p veff^2 + 2 xhat . (v @ wfc_eff + vwc) + xhat . (xhat G)
with G = wfc_eff^T wfc_eff, wsum = sum_p wfc_eff, vwc = bias_fc @ wfc_eff
precomputed on host and VW = v @ wfc_eff accumulated on device during the
DMA-bound load phase.  The 1KB AllReduce of the stats therefore overlaps the
fc matmul instead of being a serial ~38us tail (a warm-up AllReduce at kernel
start absorbs the CC entry barrier, cutting the real AR to ~12-23us), and the
BN affine is fused into the fc epilogue.

Scheduling notes (engine FIFOs are in-order):
 - the scalar (ACT) queue only issues the early activation/wv DMAs; mid-kernel
   bulk loads go on gpsimd/sync so they can't stall ACT compute.
 - the ACT head loop is exp-only (aT copies on DVE) -- mixing activation
   functions thrashes the ACT function tables.
 - S/exp/transpose head work is interleaved with the V-projection matmul
   stream so attention latency hides under the wv DMA.
 - all output stores go on the sync (HWDGE) queue; gpsimd SWDGE costs ~2us
   fixed per DMA.

Sharding: data-parallel over batch, 2 batches per core on 8 cores; BN stats
combined with a [128,2] AllReduce.

Dtypes: fp8e4 (x256 pre-scale, folded into the exp scale) for the Q/K path
and (x32) for the VW stats matmul -- both DoubleRow at 2x PE rate; bf16 for
the V / fc / residual path (the LN amplifies O errors ~20x there, fp8 fails
the 2e-2 gate; verified numerically on host).
"""

import os
from contextlib import ExitStack

import ml_dtypes
import numpy as np

import concourse.mybir as mybir
import concourse.tile as tile
from concourse import bacc
from concourse.bass_utils import run_bass_kernel_spmd
from concourse.masks import make_identity

# ---- problem constants (hardcoded per contract) ----
B, C, HH, WW = 16, 128, 64, 64
P = HH * WW           # 4096
NH, LD = 8, 64
D = NH * LD           # 512
N_CORES = 8
BPC = B // N_CORES    # 2 batches per core
NC4 = 16              # 256-row contraction chunks (DoubleRow)
LN_EPS = 1e-6
BN_EPS = 1e-5
F32 = mybir.dt.float32
BF16 = mybir.dt.bfloat16
FP8 = mybir.dt.float8e4
DR = mybir.MatmulPerfMode.DoubleRow

SC_QK = 256.0         # fp8 pre-scale for wq/wk (keeps them out of subnormals)
SC_VW = 32.0          # fp8 pre-scale for wfn in the VW stats matmul

MODE = "v3"           # printed by test.py
_ENV_KEY = lambda: (os.environ.get("V2_WARMAR", "1"), os.environ.get("V2_SKIP_COLL", "0"))
_BUILD_CACHE: dict = {}
LAST_RESULTS = None


def _emit(ctx, nc, tc, io, temp):
    AF = mybir.ActivationFunctionType
    ALU = mybir.AluOpType
    AX = mybir.AxisListType
    esc = 1.0 / (SC_QK * SC_QK * temp)   # exp arg: undo fp8 pre-scale + temperature

    consts = ctx.enter_context(tc.tile_pool(name="consts", bufs=1))
    wpool = ctx.enter_context(tc.tile_pool(name="wpool", bufs=2))
    apool = ctx.enter_context(tc.tile_pool(name="apool", bufs=1))
    res = ctx.enter_context(tc.tile_pool(name="res", bufs=1))    # resident
    sb = ctx.enter_context(tc.tile_pool(name="sb", bufs=2))
    att = ctx.enter_context(tc.tile_pool(name="att", bufs=1))
    small = ctx.enter_context(tc.tile_pool(name="small", bufs=6))
    stg = ctx.enter_context(tc.tile_pool(name="stg", bufs=1))
    tpool = ctx.enter_context(tc.tile_pool(name="tp", bufs=4))
    dram = ctx.enter_context(tc.tile_pool(name="dram", bufs=1, space="DRAM"))

    # identity for PE transposes (bf16)
    ident_f = consts.tile([128, 128], F32, tag="identf", name="identf")
    make_identity(nc, ident_f)
    ident = consts.tile([128, 128], BF16, tag="ident", name="ident")
    nc.vector.tensor_copy(out=ident, in_=ident_f)

    # ---- warm-up collective: absorbs the CC entry barrier early ----
    if os.environ.get("V2_WARMAR", "1") == "1":
        cw_in = dram.tile([128, 1], F32, tag="cw_in", name="cw_in")
        cw_out = dram.tile([128, 1], F32, tag="cw_out", name="cw_out")
        warm_sb = consts.tile([128, 1], F32, tag="warm_sb", name="warm_sb")
        nc.vector.memset(warm_sb, 0.0)
        nc.gpsimd.dma_start(out=cw_in[:, :], in_=warm_sb)
        nc.gpsimd.collective_compute(
            "AllReduce", ALU.add, replica_groups=[list(range(N_CORES))],
            ins=[cw_in.opt()], outs=[cw_out.opt()])

    bng = consts.tile([128, 1], F32, tag="bng", name="bng")
    bnb = consts.tile([128, 1], F32, tag="bnb", name="bnb")
    svin = consts.tile([128, 2 * BPC], F32, tag="svin", name="svin")
    nc.gpsimd.dma_start(out=bng, in_=io["bng"][:, :])
    nc.gpsimd.dma_start(out=bnb, in_=io["bnb"][:, :])
    nc.gpsimd.dma_start(out=svin, in_=io["svin"][:, :])

    # ---- PSUM: warm-up transpose in a throwaway pool ----
    with tc.tile_pool(name="ps_wm", bufs=1, space="PSUM") as pw:
        warm = pw.tile([128, 128], BF16, tag="warmt", name="warmt")
        nc.tensor.transpose(warm[:, :], ident[:, :], ident[:, :])

    # vT16: gpsimd queue, first -- needed by the V projection.
    vT16_sb = []
    for b in range(BPC):
        t16 = res.tile([128, 32, 128], BF16, tag=f"vT16_{b}", name=f"vT16_{b}")
        nc.gpsimd.dma_start(out=t16[:, :], in_=io["vT16"][b])
        vT16_sb.append(t16)

    # attention-era PSUM: one f32 bank (3 S slots + 2 a12 slots) and one bf16
    # transpose bank (8 slots); lives for the whole kernel (2 banks).
    # PSUM reserves a full 2KB bank per tag, so slots are hand-sliced.
    ps_at = ctx.enter_context(tc.tile_pool(name="ps_at", bufs=1, space="PSUM"))
    Sbank = ps_at.tile([128, 4, 128], F32, tag="Sbank", name="Sbank")
    Tbank = ps_at.tile([128, 8, 128], BF16, tag="Tbank", name="Tbank")
    tslot = [0]

    def tslot_next():
        s = tslot[0] % 8
        tslot[0] += 1
        return s

    ps_qkv = tc.tile_pool(name="ps_qkv", bufs=1, space="PSUM")
    pa = ps_qkv.__enter__()
    QTp = [pa.tile([128, 4, 128], F32, tag=f"QTp{b}", name=f"QTp{b}") for b in range(BPC)]
    KTp = [pa.tile([128, 4, 128], F32, tag=f"KTp{b}", name=f"KTp{b}") for b in range(BPC)]
    Vp = [pa.tile([128, D], F32, tag=f"Vp{b}", name=f"Vp{b}") for b in range(BPC)]

    # ---- phase A-QK: Q^T/K^T projections (fp8 DR), streaming ----
    # acts: 2 group-DMAs per (b, tensor) of 8 DR chunks each (256 KB), scalar q
    qgs, kgs = [], []
    for b in range(BPC):
        qg = [apool.tile([128, 8, 2, 128], FP8, tag=f"qg{b}{g}", name=f"qg{b}{g}")
              for g in range(2)]
        kg = [apool.tile([128, 8, 2, 128], FP8, tag=f"kg{b}{g}", name=f"kg{b}{g}")
              for g in range(2)]
        for g in range(2):
            nc.scalar.dma_start(out=qg[g], in_=io["qT8"][b, g])
            nc.scalar.dma_start(out=kg[g], in_=io["kT8"][b, g])
        qgs.append(qg); kgs.append(kg)
    # weights: 4 group-DMAs per tensor of 4 DR chunks each (512 KB), sync q
    for g in range(4):
        wq_c = wpool.tile([128, 4, 2, D], FP8, tag="wq_c", name="wq_c")
        wk_c = wpool.tile([128, 4, 2, D], FP8, tag="wk_c", name="wk_c")
        nc.sync.dma_start(out=wq_c, in_=io["wq8"][g])
        nc.sync.dma_start(out=wk_c, in_=io["wk8"][g])
        for sub in range(4):
            pc = 4 * g + sub
            for b in range(BPC):
                qc = qgs[b][pc // 8][:, pc % 8, :, :]
                kc = kgs[b][pc // 8][:, pc % 8, :, :]
                for db in range(4):
                    # one PSUM accumulation group per bank: start only on the
                    # first write into the bank, stop on the very last
                    st = pc == 0 and db == 0
                    sp = pc == NC4 - 1 and db == 3
                    nc.tensor.matmul(QTp[b][:, db, :],
                                     wq_c[:, sub, :, db * 128:(db + 1) * 128],
                                     qc, start=st, stop=sp, perf_mode=DR)
                    nc.tensor.matmul(KTp[b][:, db, :],
                                     wk_c[:, sub, :, db * 128:(db + 1) * 128],
                                     kc, start=st, stop=sp, perf_mode=DR)

    # VW inputs behind the wq/wk stream on sync (needed ~mid-kernel)
    vT8_sb = []
    for b in range(BPC):
        t8 = res.tile([128, NC4, 2, 128], FP8, tag=f"vT8_{b}", name=f"vT8_{b}")
        nc.sync.dma_start(out=t8[:, :], in_=io["vT8"][b])
        vT8_sb.append(t8)
    wfn_sb = res.tile([128, NC4, 2, D], FP8, tag="wfn", name="wfn")
    for g in range(2):
        nc.sync.dma_start(out=wfn_sb[:, 8 * g:8 * g + 8], in_=io["wfn"][:, 8 * g:8 * g + 8])
    G_sb = res.tile([128, 4, D], BF16, tag="G", name="G")
    nc.scalar.dma_start(out=G_sb[:, :], in_=io["G"][:, :])
    wv2 = res.tile([128, 4, 2], BF16, tag="wv2", name="wv2")
    nc.scalar.dma_start(out=wv2[:, :], in_=io["wv2"][:, :])

    # residual + fc weights: gpsimd queue, behind vT16
    veff_sb = []
    for b in range(BPC):
        t = res.tile([128, P], BF16, tag=f"veff{b}", name=f"veff{b}")
        nc.gpsimd.dma_start(out=t[:, :], in_=io["veff"][b])
        veff_sb.append(t)
    wfcT = res.tile([128, 4, P], BF16, tag="wfcT", name="wfcT")
    for dc in range(4):
        nc.gpsimd.dma_start(out=wfcT[:, dc], in_=io["wfcT"][dc])

    # ---- evacuate QT/KT to SBUF (bf16) ----
    qkv_sb = []
    for b in range(BPC):
        QT_sb = sb.tile([128, 4, 128], BF16, tag="QT_sb", name="QT_sb")
        KT_sb = sb.tile([128, 4, 128], BF16, tag="KT_sb", name="KT_sb")
        nc.vector.tensor_copy(out=QT_sb, in_=QTp[b][:, :, :])
        nc.scalar.copy(out=KT_sb, in_=KTp[b][:, :, :])
        qkv_sb.append([QT_sb, KT_sb, None])

    # ---- phase A-V (bf16 stream) interleaved with S/exp head work ----
    # heads: (b, h) pairs; emit 4 per wv group BEFORE that group's V matmuls
    heads = [(b, h) for h in range(NH) for b in range(BPC)]
    aTs = {}
    rss = {}
    for g in range(4):
        for (b, h) in heads[4 * g:4 * g + 4]:
            QT_sb, KT_sb, _ = qkv_sb[b]
            po = (h % 2) * 64
            dc = h // 2
            S = Sbank[:, (2 * b + h) % 3, :]
            nc.tensor.matmul(S, QT_sb[po:po + 64, dc, :], KT_sb[po:po + 64, dc, :],
                             start=True, stop=True)
            e_f = sb.tile([128, 128], BF16, tag="e_f", name="e_f")
            lsum = small.tile([128, 1], F32, tag="lsum", name="lsum")
            nc.scalar.activation(out=e_f, in_=S, func=AF.Exp, scale=esc,
                                 accum_out=lsum)
            rs = small.tile([128, 1], F32, tag="rs", name="rs")
            nc.vector.reciprocal(rs, lsum)
            tpa = Tbank[:, tslot_next(), :]
            nc.tensor.transpose(tpa, e_f[:, :], ident[:, :])
            aT = att.tile([128, 128], BF16, tag=f"aT{b}{h}", name=f"aT{b}{h}")
            nc.vector.tensor_copy(out=aT, in_=tpa)
            aTs[(b, h)] = aT
            rss[(b, h)] = rs
        wv_c = wpool.tile([128, 8, D], BF16, tag="wv_c", name="wv_c")
        nc.scalar.dma_start(out=wv_c, in_=io["wv16"][g])
        for sub in range(8):
            pc2 = 8 * g + sub
            for b in range(BPC):
                nc.tensor.matmul(Vp[b][:, :], vT16_sb[b][:, pc2, :],
                                 wv_c[:, sub, :],
                                 start=pc2 == 0, stop=pc2 == 31)

    # evacuate V, free QKV PSUM banks
    for b in range(BPC):
        V_sb = sb.tile([128, D], BF16, tag="V_sb", name="V_sb")
        nc.vector.tensor_copy(out=V_sb, in_=Vp[b][:, :])
        qkv_sb[b][2] = V_sb
    ps_qkv.__exit__(None, None, None)

    # post-phase-A PSUM pools (2 banks each): VW accumulators, O banks, fc
    ps_vw = ctx.enter_context(tc.tile_pool(name="ps_vw", bufs=1, space="PSUM"))
    VWp = [ps_vw.tile([128, D], F32, tag=f"VWp{b}", name=f"VWp{b}") for b in range(BPC)]
    ps_ao = ctx.enter_context(tc.tile_pool(name="ps_ao", bufs=1, space="PSUM"))
    Obank = [ps_ao.tile([128, D], F32, tag=f"Ob{b}", name=f"Ob{b}") for b in range(BPC)]
    ps_fc = ctx.enter_context(tc.tile_pool(name="ps_fc", bufs=2, space="PSUM"))

    # ---- A@V for all heads ----
    Oscs = []
    for b in range(BPC):
        Osc = sb.tile([128, D], F32, tag="Osc", name="Osc")
        Oscs.append(Osc)
    for (b, h) in heads:
        nc.tensor.matmul(Obank[b][:, h * 64:(h + 1) * 64], aTs[(b, h)][:, :],
                         qkv_sb[b][2][:, h * 64:(h + 1) * 64], start=True, stop=True)
        nc.vector.tensor_scalar_mul(out=Oscs[b][:, h * 64:(h + 1) * 64],
                                    in0=Obank[b][:, h * 64:(h + 1) * 64],
                                    scalar1=rss[(b, h)])

    # ---- VW = (v @ wfc_eff) * SC_VW via fp8 DR, accumulating ----
    for pc in range(NC4):
        for b in range(BPC):
            nc.tensor.matmul(VWp[b][:, :], vT8_sb[b][:, pc, :, :], wfn_sb[:, pc, :, :],
                             start=pc == 0, stop=pc == NC4 - 1, perf_mode=DR)

    # ---- silu + LN + stats per batch ----
    xTs = []
    cin_sb = small.tile([128, 2], F32, tag="cin_sb", name="cin_sb")
    s1l = [small.tile([128, 1], F32, tag=f"s1l{b}", name=f"s1l{b}") for b in range(BPC)]
    s2l = [small.tile([128, 1], F32, tag=f"s2l{b}", name=f"s2l{b}") for b in range(BPC)]
    for b in range(BPC):
        Osc = Oscs[b]
        sg = sb.tile([128, D], F32, tag="sg", name="sg")
        nc.scalar.activation(out=sg, in_=Osc, func=AF.Sigmoid)
        Osw = sb.tile([128, D], F32, tag="Osw", name="Osw")
        nc.vector.tensor_mul(out=Osw, in0=Osc, in1=sg)
        st6 = small.tile([128, 6], F32, tag="st6", name="st6")
        nc.vector.bn_stats(out=st6, in_=Osw)
        mv = small.tile([128, 2], F32, tag="mv", name="mv")
        nc.vector.bn_aggr(out=mv, in_=st6)
        sd = small.tile([128, 1], F32, tag="sd", name="sd")
        nc.scalar.activation(out=sd, in_=mv[:, 1:2], func=AF.Sqrt,
                             scale=float(D) / (D - 1))
        nc.vector.tensor_scalar_add(out=sd, in0=sd, scalar1=LN_EPS)
        rstd = small.tile([128, 1], F32, tag="rstd", name="rstd")
        nc.vector.reciprocal(rstd, sd)
        xhat = sb.tile([128, D], BF16, tag="xhat", name="xhat")
        nc.vector.tensor_scalar(out=xhat, in0=Osw, scalar1=mv[:, 0:1], scalar2=rstd,
                                op0=ALU.subtract, op1=ALU.mult)
        xT = sb.tile([128, 4, 128], BF16, tag="xT", name="xT")
        for dc in range(4):
            tp = Tbank[:, tslot_next(), :]
            nc.tensor.transpose(tp, xhat[:, dc * 128:(dc + 1) * 128], ident[:, :])
            nc.vector.tensor_copy(out=xT[:, dc, :], in_=tp)
        xTs.append(xT)

        # stats: S1 = sv1 + xhat.wsum ; S2 = sv2 + (2/SC)xhat.VW + 2 xhat.vwc + xhat.(xhat G)
        # Zp reuses the (now idle) O bank so it doesn't couple into the fc
        # PSUM rotation and stall fc matmuls on the stats reads.
        Zp = Obank[b]
        a12 = Sbank[:, 3, 2 * b:2 * b + 2]
        for dc in range(4):
            nc.tensor.matmul(Zp[:, :], xT[:, dc, :], G_sb[:, dc, :],
                             start=dc == 0, stop=dc == 3)
            nc.tensor.matmul(a12, xT[:, dc, :], wv2[:, dc, :],
                             start=dc == 0, stop=dc == 3)
        j1 = tpool.tile([128, D], BF16, tag="junk", name="junk1")
        j2 = tpool.tile([128, D], BF16, tag="junk", name="junk2")
        nc.vector.tensor_mul(out=j1, in0=xhat, in1=VWp[b][:, :])
        nc.vector.tensor_mul(out=j2, in0=xhat, in1=Zp[:, :])
        r1 = small.tile([128, 1], F32, tag="r1", name="r1")
        r2 = small.tile([128, 1], F32, tag="r2", name="r2")
        nc.vector.reduce_sum(r1, j1, axis=AX.X)
        nc.vector.reduce_sum(r2, j2, axis=AX.X)
        s2a = small.tile([128, 1], F32, tag="s2a", name="s2a")
        nc.vector.tensor_scalar(out=s2a, in0=r1, scalar1=2.0 / SC_VW,
                                scalar2=svin[:, 2 * b + 1:2 * b + 2],
                                op0=ALU.mult, op1=ALU.add)
        s2b = small.tile([128, 1], F32, tag="s2b", name="s2b")
        nc.vector.tensor_add(out=s2b, in0=s2a, in1=r2)
        nc.vector.tensor_add(out=s2l[b], in0=s2b, in1=a12[:, 1:2])
        nc.vector.tensor_add(out=s1l[b], in0=svin[:, 2 * b:2 * b + 1], in1=a12[:, 0:1])
    nc.vector.tensor_add(out=cin_sb[:, 0:1], in0=s1l[0], in1=s1l[1])
    nc.vector.tensor_add(out=cin_sb[:, 1:2], in0=s2l[0], in1=s2l[1])

    # ---- stats AllReduce (overlaps the fc phase below) ----
    cin = dram.tile([128, 2], F32, tag="cin", name="cin")
    cout = dram.tile([128, 2], F32, tag="cout", name="cout")
    nc.gpsimd.dma_start(out=cin[:, :], in_=cin_sb)
    if os.environ.get("V2_SKIP_COLL", "0") == "1":
        nc.gpsimd.dma_start(out=cout[:, :], in_=cin[:, :])
    else:
        nc.gpsimd.collective_compute(
            "AllReduce", ALU.add, replica_groups=[list(range(N_CORES))],
            ins=[cin.opt()], outs=[cout.opt()])
    red = small.tile([128, 2], F32, tag="red", name="red")
    nc.gpsimd.dma_start(out=red[:, :], in_=cout[:, :])

    # ---- fc matmuls + residual add (not AR-gated) ----
    tsegs = []
    for pt in range(8):
        for b in range(BPC):
            O2 = ps_fc.tile([128, 512], F32, tag="O2", name="O2")
            for dc in range(4):
                nc.tensor.matmul(O2[:, :], xTs[b][:, dc, :],
                                 wfcT[:, dc, pt * 512:(pt + 1) * 512],
                                 start=dc == 0, stop=dc == 3)
            tseg = tpool.tile([128, 512], BF16, tag="tseg", name="tseg")
            nc.vector.tensor_add(out=tseg, in0=veff_sb[b][:, pt * 512:(pt + 1) * 512],
                                 in1=O2[:, :])
            tsegs.append((b, pt, tseg))

    # ---- post-AR: BN affine factors ----
    inv_n = 1.0 / float(B * P)
    mean = small.tile([128, 1], F32, tag="mean", name="mean")
    nc.scalar.mul(out=mean, in_=red[:, 0:1], mul=inv_n)
    ex2 = small.tile([128, 1], F32, tag="ex2", name="ex2")
    nc.scalar.mul(out=ex2, in_=red[:, 1:2], mul=inv_n)
    msq = small.tile([128, 1], F32, tag="msq", name="msq")
    nc.vector.tensor_mul(out=msq, in0=mean, in1=mean)
    var = small.tile([128, 1], F32, tag="var", name="var")
    nc.vector.tensor_sub(out=var, in0=ex2, in1=msq)
    epsbn = consts.tile([128, 1], F32, tag="epsbn", name="epsbn")
    nc.vector.memset(epsbn, BN_EPS)
    sdv = small.tile([128, 1], F32, tag="sdv", name="sdv")
    nc.scalar.activation(out=sdv, in_=var, func=AF.Sqrt, bias=epsbn)
    invs = small.tile([128, 1], F32, tag="invs", name="invs")
    nc.vector.reciprocal(invs, sdv)
    scl = small.tile([128, 1], F32, tag="scl", name="scl")
    nc.vector.tensor_mul(out=scl, in0=bng, in1=invs)
    tmp = small.tile([128, 1], F32, tag="tmp", name="tmp")
    nc.vector.tensor_mul(out=tmp, in0=mean, in1=scl)
    shf = small.tile([128, 1], F32, tag="shf", name="shf")
    nc.vector.tensor_sub(out=shf, in0=bnb, in1=tmp)

    # ---- AR-gated epilogue: scale+shift into per-batch staging tiles (ACT
    # handles even pt, DVE odd pt), then one big store per batch on separate
    # HWDGE queues (per-segment stores serialize ~1us each on one ring) ----
    stages = [stg.tile([128, P], BF16, tag=f"stage{b}", name=f"stage{b}")
              for b in range(BPC)]
    for i, (b, pt, tseg) in enumerate(tsegs):
        out_sl = stages[b][:, pt * 512:(pt + 1) * 512]
        if pt % 2 == 0:
            nc.scalar.activation(out=out_sl, in_=tseg, func=AF.Identity,
                                 bias=shf, scale=scl)
        else:
            nc.vector.tensor_scalar(out=out_sl, in0=tseg, scalar1=scl, scalar2=shf,
                                    op0=ALU.mult, op1=ALU.add)
    nc.sync.dma_start(out=io["out"][0], in_=stages[0][:, :])
    nc.scalar.dma_start(out=io["out"][1], in_=stages[1][:, :])


def _build(temp):
    key = (MODE, temp, _ENV_KEY())
    if key in _BUILD_CACHE:
        return _BUILD_CACHE[key]
    nc = bacc.Bacc("TRN2", target_bir_lowering=False, debug=False, num_devices=N_CORES)
    io = {
        "qT8": nc.dram_tensor("qT8", [BPC, 2, 128, 8, 2, 128], FP8, kind="ExternalInput").ap(),
        "kT8": nc.dram_tensor("kT8", [BPC, 2, 128, 8, 2, 128], FP8, kind="ExternalInput").ap(),
        "vT8": nc.dram_tensor("vT8", [BPC, 128, NC4, 2, 128], FP8, kind="ExternalInput").ap(),
        "vT16": nc.dram_tensor("vT16", [BPC, 128, 32, 128], BF16, kind="ExternalInput").ap(),
        "veff": nc.dram_tensor("veff", [BPC, C, P], BF16, kind="ExternalInput").ap(),
        "wq8": nc.dram_tensor("wq8", [4, 128, 4, 2, D], FP8, kind="ExternalInput").ap(),
        "wk8": nc.dram_tensor("wk8", [4, 128, 4, 2, D], FP8, kind="ExternalInput").ap(),
        "wv16": nc.dram_tensor("wv16", [4, 128, 8, D], BF16, kind="ExternalInput").ap(),
        "wfn": nc.dram_tensor("wfn", [128, NC4, 2, D], FP8, kind="ExternalInput").ap(),
        "wfcT": nc.dram_tensor("wfcT", [4, 128, P], BF16, kind="ExternalInput").ap(),
        "G": nc.dram_tensor("G", [128, 4, D], BF16, kind="ExternalInput").ap(),
        "wv2": nc.dram_tensor("wv2", [128, 4, 2], BF16, kind="ExternalInput").ap(),
        "svin": nc.dram_tensor("svin", [C, 2 * BPC], F32, kind="ExternalInput").ap(),
        "bng": nc.dram_tensor("bng", [C, 1], F32, kind="ExternalInput").ap(),
        "bnb": nc.dram_tensor("bnb", [C, 1], F32, kind="ExternalInput").ap(),
        "out": nc.dram_tensor("out", [BPC, C, P], BF16, kind="ExternalOutput").ap(),
    }
    with tile.TileContext(nc) as tc, ExitStack() as ctx:
        _emit(ctx, nc, tc, io, temp)
    nc.compile()
    _BUILD_CACHE[key] = nc
    return nc


def _pack_dr_g(xT, ng, sub):
    # [P, M] -> [ng, 128, sub, 2, M]: group g, partition r, chunk sub, half i
    # covers row p = (g*sub + s)*256 + i*128 + r
    M = xT.shape[1]
    return np.ascontiguousarray(
        xT.reshape(ng, sub, 2, 128, M).transpose(0, 3, 1, 2, 4))


def _f8(x):
    return np.ascontiguousarray(np.asarray(x, np.float32).astype(ml_dtypes.float8_e4m3))


def _b16(x):
    return np.ascontiguousarray(np.asarray(x, np.float32).astype(ml_dtypes.bfloat16))


def kernel(v, k, q, w_qs, w_ks, w_vs, w_fc, ln_gamma, ln_beta, temperature,
           bn_gamma, bn_beta, **_ignored):
    v = np.asarray(v, np.float32)
    k = np.asarray(k, np.float32)
    q = np.asarray(q, np.float32)
    w_qs = np.asarray(w_qs, np.float32)
    w_ks = np.asarray(w_ks, np.float32)
    w_vs = np.asarray(w_vs, np.float32)
    w_fc = np.asarray(w_fc, np.float32)
    ln_gamma = np.asarray(ln_gamma, np.float32)
    ln_beta = np.asarray(ln_beta, np.float32)
    temp = float(np.asarray(temperature))
    bn_gamma = np.asarray(bn_gamma, np.float32)
    bn_beta = np.asarray(bn_beta, np.float32)

    qf = q.reshape(B, C, P)
    kf = k.reshape(B, C, P)
    vf = v.reshape(B, C, P)

    # activations: [P, C] transposed, group-DR-packed for q/k; r-major for vT8/vT16
    qT8 = np.stack([_f8(_pack_dr_g(qf[b].T, 2, 8)) for b in range(B)])
    kT8 = np.stack([_f8(_pack_dr_g(kf[b].T, 2, 8)) for b in range(B)])
    # vT8 SBUF layout [r, pc, i, c]:
    vT8 = np.stack([_f8(vf[b].T.reshape(NC4, 2, 128, C).transpose(2, 0, 1, 3))
                    for b in range(B)])
    # vT16 SBUF layout [r, pc2, c]:
    vT16 = np.stack([_b16(vf[b].T.reshape(32, 128, C).transpose(1, 0, 2))
                     for b in range(B)])

    # weights
    wq8 = _f8(_pack_dr_g(w_qs.T * SC_QK, 4, 4))
    wk8 = _f8(_pack_dr_g(w_ks.T * SC_QK, 4, 4))
    wv16 = _b16(w_vs.T.reshape(4, 8, 128, D).transpose(0, 2, 1, 3))
    wfc_eff = (w_fc * ln_gamma[None, :]).astype(np.float32)       # [P, D]
    # wfn SBUF layout [r, pc, i, d]:
    wfn8 = _f8((wfc_eff * SC_VW).reshape(NC4, 2, 128, D).transpose(2, 0, 1, 3))
    wfcT16 = _b16(wfc_eff.T.reshape(4, 128, P))
    G = (wfc_eff.T.astype(np.float64) @ wfc_eff.astype(np.float64)).astype(np.float32)
    G16 = _b16(G.reshape(4, 128, D).transpose(1, 0, 2))           # [r, dc, d]
    bias_fc = (w_fc @ ln_beta).astype(np.float32)                 # [P]
    wsum = wfc_eff.sum(0)
    vwc = (bias_fc.astype(np.float64) @ wfc_eff.astype(np.float64)).astype(np.float32)
    wv2 = _b16(np.stack([wsum, 2.0 * vwc], axis=1)
               .reshape(4, 128, 2).transpose(1, 0, 2))            # [r, dc, j]

    veff_f = vf + bias_fc[None, None, :]
    veff = _b16(veff_f)
    # per-batch input-residual partial sums for the BN stats decomposition
    sv1 = veff_f.astype(np.float64).sum(-1)                       # [B, C]
    sv2 = (veff_f.astype(np.float64) ** 2).sum(-1)
    svin_all = np.stack([sv1, sv2], axis=-1).astype(np.float32)   # [B, C, 2]

    bng = np.ascontiguousarray(bn_gamma.reshape(C, 1))
    bnb = np.ascontiguousarray(bn_beta.reshape(C, 1))

    nc = _build(temp)
    in_maps = []
    for i in range(N_CORES):
        bs = slice(BPC * i, BPC * (i + 1))
        # svin layout: [c, 2*b + (0:sum, 1:sumsq)]
        svin = np.ascontiguousarray(
            svin_all[bs].transpose(1, 0, 2).reshape(C, 2 * BPC))
        in_maps.append({
            "qT8": qT8[bs], "kT8": kT8[bs], "vT8": vT8[bs], "vT16": vT16[bs],
            "veff": veff[bs], "wq8": wq8, "wk8": wk8, "wv16": wv16,
            "wfn": wfn8, "wfcT": wfcT16, "G": G16, "wv2": wv2,
            "svin": svin, "bng": bng, "bnb": bnb,
        })
    res = run_bass_kernel_spmd(nc, in_maps, core_ids=list(range(N_CORES)))
    global LAST_RESULTS
    LAST_RESULTS = res
    out = np.concatenate([np.asarray(res.results[i]["out"], np.float32)
                          for i in range(N_CORES)], axis=0)
    return out.reshape(B, C, HH, WW)


# revision 48
# speedup vs baseline: 1.3469x; 1.0654x over previous
"""Trainium2 Bass kernel for nn_MultiHeadAttention (channel-attention block).

Math per batch (X* = reshape(*, [C,P]), P=4096, C=128, D=512, 8 heads x 64):
  Q^T = Wq^T Xq^T, K^T = Wk^T Xk^T   (computed directly transposed, fp8 DR)
  V   = Xv Wv^T                      (bf16)
  per head: e = exp(Q_h K_h^T * esc); O_h = (e / rowsum(e)) V_h
  O = silu(O); xhat = (O - mean)/(unbiased_std + eps)   (LN affine folded
  into wfc_eff = w_fc * ln_gamma and veff = v + w_fc @ ln_beta)
  out_pre = veff + xhat @ wfc_eff^T
  out = BatchNorm2d(out_pre), batch stats over (b,h,w)

BN statistics are computed BEFORE the fc matmul via the decomposition
  sum_p out   = sum_p veff + xhat . wsum
  sum_p out^2 = sum_p veff^2 + 2 xhat . (v @ wfc_eff + vwc) + xhat . (xhat G)
with G = wfc_eff^T wfc_eff, wsum = sum_p wfc_eff, vwc = bias_fc @ wfc_eff
precomputed on host and VW = v @ wfc_eff accumulated on device.  The 1KB
AllReduce of the stats therefore overlaps the fc matmul instead of being a
serial ~38us tail (a warm-up AllReduce at kernel start absorbs the CC entry
barrier, cutting the real AR to ~12-23us), and the BN affine is fused into
the fc epilogue.

Scheduling notes (engine FIFOs are in-order; HWDGE queues = sync/scalar,
gpsimd SWDGE drains at only ~90 GB/s so it carries the smallest share):
 - critical act/weight DMAs are the first instructions on their queues
 - the ACT head loop is exp-only (aT copies on DVE) -- mixing activation
   functions thrashes the ACT function tables
 - S/exp/transpose head work is interleaved with the V-projection matmul
   stream so attention latency hides under the wv DMA
 - PE order: QK -> S/V interleave -> A@V -> xT transposes -> VW -> stats
   matmuls -> fc, so the stats AllReduce triggers as early as possible
 - fc runs in fp8 DoubleRow: wfcT8 = fp8(32*wfc^T), xhat8 = fp8(xhat),
   veff pre-scaled by 32 on host, and scl/32 folded into the BN affine

Sharding: data-parallel over batch, 2 batches per core on 8 cores; BN stats
combined with a [128,2] AllReduce.

Dtypes: fp8e4 (x256 pre-scale folded into the exp scale) for the Q/K path,
(x32) for VW and fc -- all DoubleRow at 2x PE rate; bf16 for the V path and
residual (the LN amplifies O errors ~20x there, fp8 fails the 2e-2 gate;
verified numerically on host).
"""

import os
from contextlib import ExitStack

import ml_dtypes
import numpy as np

import concourse.mybir as mybir
import concourse.tile as tile
from concourse import bacc
from concourse.bass_utils import run_bass_kernel_spmd
from concourse.masks import make_identity

# ---- problem constants (hardcoded per contract) ----
B, C, HH, WW = 16, 128, 64, 64
P = HH * WW           # 4096
NH, LD = 8, 64
D = NH * LD           # 512
N_CORES = 8
BPC = B // N_CORES    # 2 batches per core
NC4 = 16              # 256-row contraction chunks (DoubleRow)
LN_EPS = 1e-6
BN_EPS = 1e-5
F32 = mybir.dt.float32
BF16 = mybir.dt.bfloat16
FP8 = mybir.dt.float8e4
DR = mybir.MatmulPerfMode.DoubleRow

SC_QK = 256.0         # fp8 pre-scale for wq/wk (keeps them out of subnormals)
SC_VW = 32.0          # fp8 pre-scale for wfn in the VW stats matmul
SC_FC = 32.0          # fp8 pre-scale for wfcT8 (veff host-scaled to match)

MODE = "v5"           # printed by test.py
_ENV_KEY = lambda: (os.environ.get("V2_WARMAR", "1"), os.environ.get("V2_SKIP_COLL", "0"))
_BUILD_CACHE: dict = {}
LAST_RESULTS = None


def _emit(ctx, nc, tc, io, temp):
    AF = mybir.ActivationFunctionType
    ALU = mybir.AluOpType
    AX = mybir.AxisListType
    esc = 1.0 / (SC_QK * SC_QK * temp)   # exp arg: undo fp8 pre-scale + temperature

    consts = ctx.enter_context(tc.tile_pool(name="consts", bufs=1))
    wpool = ctx.enter_context(tc.tile_pool(name="wpool", bufs=2))
    apool = ctx.enter_context(tc.tile_pool(name="apool", bufs=1))
    res = ctx.enter_context(tc.tile_pool(name="res", bufs=1))    # resident
    sb = ctx.enter_context(tc.tile_pool(name="sb", bufs=2))
    att = ctx.enter_context(tc.tile_pool(name="att", bufs=1))
    small = ctx.enter_context(tc.tile_pool(name="small", bufs=6))
    stg = ctx.enter_context(tc.tile_pool(name="stg", bufs=1))
    tpool = ctx.enter_context(tc.tile_pool(name="tp", bufs=4))
    dram = ctx.enter_context(tc.tile_pool(name="dram", bufs=1, space="DRAM"))

    # ---- critical input DMAs first on each engine FIFO ----
    # scalar (HWDGE): q/k activations, then wv16 groups later in the V loop
    qgs, kgs = [], []
    for b in range(BPC):
        qg = [apool.tile([128, 8, 2, 128], FP8, tag=f"qg{b}{g}", name=f"qg{b}{g}")
              for g in range(2)]
        kg = [apool.tile([128, 8, 2, 128], FP8, tag=f"kg{b}{g}", name=f"kg{b}{g}")
              for g in range(2)]
        for g in range(2):
            nc.scalar.dma_start(out=qg[g], in_=io["qT8"][b, g])
            nc.scalar.dma_start(out=kg[g], in_=io["kT8"][b, g])
        qgs.append(qg); kgs.append(kg)
    # sync (HWDGE): vT16 b1 first (V proj needs it by ~40us), then wq/wk
    # groups (inside the QK loop), then vT8/wfn
    vT16_sb = [res.tile([128, 32, 128], BF16, tag=f"vT16_{b}", name=f"vT16_{b}")
               for b in range(BPC)]
    nc.sync.dma_start(out=vT16_sb[1][:, :], in_=io["vT16"][1])
    # gpsimd (SWDGE, slow ~90GB/s): smallest share: vT16 b0, veff, misc
    nc.gpsimd.dma_start(out=vT16_sb[0][:, :], in_=io["vT16"][0])

    bng = consts.tile([128, 1], F32, tag="bng", name="bng")
    bnb = consts.tile([128, 1], F32, tag="bnb", name="bnb")
    svin = consts.tile([128, 2 * BPC], F32, tag="svin", name="svin")
    nc.gpsimd.dma_start(out=bng, in_=io["bng"][:, :])
    nc.gpsimd.dma_start(out=bnb, in_=io["bnb"][:, :])
    nc.gpsimd.dma_start(out=svin, in_=io["svin"][:, :])

    # ---- warm-up collective: absorbs the CC entry barrier early ----
    if os.environ.get("V2_WARMAR", "1") == "1":
        cw_in = dram.tile([128, 1], F32, tag="cw_in", name="cw_in")
        cw_out = dram.tile([128, 1], F32, tag="cw_out", name="cw_out")
        warm_sb = consts.tile([128, 1], F32, tag="warm_sb", name="warm_sb")
        nc.vector.memset(warm_sb, 0.0)
        nc.gpsimd.dma_start(out=cw_in[:, :], in_=warm_sb)
        nc.gpsimd.collective_compute(
            "AllReduce", ALU.add, replica_groups=[list(range(N_CORES))],
            ins=[cw_in.opt()], outs=[cw_out.opt()])

    # residual (host-scaled by SC_FC to match the fp8 fc output)
    veff_sb = []
    for b in range(BPC):
        t = res.tile([128, P], BF16, tag=f"veff{b}", name=f"veff{b}")
        nc.gpsimd.dma_start(out=t[:, :], in_=io["veff"][b])
        veff_sb.append(t)

    # identity for PE transposes (bf16)
    ident_f = consts.tile([128, 128], F32, tag="identf", name="identf")
    make_identity(nc, ident_f)
    ident = consts.tile([128, 128], BF16, tag="ident", name="ident")
    nc.vector.tensor_copy(out=ident, in_=ident_f)

    # ---- PSUM: warm-up transpose in a throwaway pool ----
    with tc.tile_pool(name="ps_wm", bufs=1, space="PSUM") as pw:
        warm = pw.tile([128, 128], BF16, tag="warmt", name="warmt")
        nc.tensor.transpose(warm[:, :], ident[:, :], ident[:, :])

    # attention-era PSUM: one f32 bank (3 S slots + 2 a12 slots) and one bf16
    # transpose bank (8 slots); lives for the whole kernel (2 banks).
    # PSUM reserves a full 2KB bank per tag, so slots are hand-sliced.
    ps_at = ctx.enter_context(tc.tile_pool(name="ps_at", bufs=1, space="PSUM"))
    Sbank = ps_at.tile([128, 4, 128], F32, tag="Sbank", name="Sbank")
    Tbank = ps_at.tile([128, 8, 128], BF16, tag="Tbank", name="Tbank")
    tslot = [0]

    def tslot_next():
        s = tslot[0] % 8
        tslot[0] += 1
        return s

    ps_qkv = tc.tile_pool(name="ps_qkv", bufs=1, space="PSUM")
    pa = ps_qkv.__enter__()
    QTp = [pa.tile([128, 4, 128], F32, tag=f"QTp{b}", name=f"QTp{b}") for b in range(BPC)]
    KTp = [pa.tile([128, 4, 128], F32, tag=f"KTp{b}", name=f"KTp{b}") for b in range(BPC)]
    Vp = [pa.tile([128, D], F32, tag=f"Vp{b}", name=f"Vp{b}") for b in range(BPC)]

    # ---- phase A-QK: Q^T/K^T projections (fp8 DR), streaming weights ----
    for g in range(4):
        wq_c = wpool.tile([128, 4, 2, D], FP8, tag="wq_c", name="wq_c")
        wk_c = wpool.tile([128, 4, 2, D], FP8, tag="wk_c", name="wk_c")
        nc.sync.dma_start(out=wq_c, in_=io["wq8"][g])
        nc.sync.dma_start(out=wk_c, in_=io["wk8"][g])
        for sub in range(4):
            pc = 4 * g + sub
            for b in range(BPC):
                qc = qgs[b][pc // 8][:, pc % 8, :, :]
                kc = kgs[b][pc // 8][:, pc % 8, :, :]
                for db in range(4):
                    # one PSUM accumulation group per bank: start only on the
                    # first write into the bank, stop on the very last
                    st = pc == 0 and db == 0
                    sp = pc == NC4 - 1 and db == 3
                    nc.tensor.matmul(QTp[b][:, db, :],
                                     wq_c[:, sub, :, db * 128:(db + 1) * 128],
                                     qc, start=st, stop=sp, perf_mode=DR)
                    nc.tensor.matmul(KTp[b][:, db, :],
                                     wk_c[:, sub, :, db * 128:(db + 1) * 128],
                                     kc, start=st, stop=sp, perf_mode=DR)

    # VW + fc inputs behind the wq/wk stream on sync (needed ~mid-kernel)
    vT8_sb = []
    for b in range(BPC):
        t8 = res.tile([128, NC4, 2, 128], FP8, tag=f"vT8_{b}", name=f"vT8_{b}")
        nc.sync.dma_start(out=t8[:, :], in_=io["vT8"][b])
        vT8_sb.append(t8)
    wfn_sb = res.tile([128, NC4, 2, D], FP8, tag="wfn", name="wfn")
    for g in range(2):
        nc.sync.dma_start(out=wfn_sb[:, 8 * g:8 * g + 8], in_=io["wfn"][:, 8 * g:8 * g + 8])

    # ---- evacuate QT/KT to SBUF (bf16) ----
    qkv_sb = []
    for b in range(BPC):
        QT_sb = sb.tile([128, 4, 128], BF16, tag="QT_sb", name="QT_sb")
        KT_sb = sb.tile([128, 4, 128], BF16, tag="KT_sb", name="KT_sb")
        nc.vector.tensor_copy(out=QT_sb, in_=QTp[b][:, :, :])
        nc.scalar.copy(out=KT_sb, in_=KTp[b][:, :, :])
        qkv_sb.append([QT_sb, KT_sb, None])

    # ---- phase A-V (bf16 stream) interleaved with S/exp head work ----
    # heads: (b, h) pairs; emit 4 per wv group BEFORE that group's V matmuls
    heads = [(b, h) for h in range(NH) for b in range(BPC)]
    aTs = {}
    rss = {}
    for g in range(4):
        for (b, h) in heads[4 * g:4 * g + 4]:
            QT_sb, KT_sb, _ = qkv_sb[b]
            po = (h % 2) * 64
            dc = h // 2
            S = Sbank[:, (2 * b + h) % 3, :]
            nc.tensor.matmul(S, QT_sb[po:po + 64, dc, :], KT_sb[po:po + 64, dc, :],
                             start=True, stop=True)
            e_f = sb.tile([128, 128], BF16, tag="e_f", name="e_f")
            lsum = small.tile([128, 1], F32, tag="lsum", name="lsum")
            nc.scalar.activation(out=e_f, in_=S, func=AF.Exp, scale=esc,
                                 accum_out=lsum)
            rs = small.tile([128, 1], F32, tag="rs", name="rs")
            nc.vector.reciprocal(rs, lsum)
            tpa = Tbank[:, tslot_next(), :]
            nc.tensor.transpose(tpa, e_f[:, :], ident[:, :])
            aT = att.tile([128, 128], BF16, tag=f"aT{b}{h}", name=f"aT{b}{h}")
            nc.vector.tensor_copy(out=aT, in_=tpa)
            aTs[(b, h)] = aT
            rss[(b, h)] = rs
        wv_c = wpool.tile([128, 8, D], BF16, tag="wv_c", name="wv_c")
        nc.scalar.dma_start(out=wv_c, in_=io["wv16"][g])
        for sub in range(8):
            pc2 = 8 * g + sub
            for b in range(BPC):
                nc.tensor.matmul(Vp[b][:, :], vT16_sb[b][:, pc2, :],
                                 wv_c[:, sub, :],
                                 start=pc2 == 0, stop=pc2 == 31)

    # stats + fc weights at the back of the scalar queue
    G_sb = res.tile([128, 4, D], BF16, tag="G", name="G")
    nc.scalar.dma_start(out=G_sb[:, :], in_=io["G"][:, :])
    wv2 = res.tile([128, 4, 2], BF16, tag="wv2", name="wv2")
    nc.scalar.dma_start(out=wv2[:, :], in_=io["wv2"][:, :])
    wfcT8 = res.tile([128, 2, 2, P], FP8, tag="wfcT8", name="wfcT8")
    for j in range(2):
        nc.scalar.dma_start(out=wfcT8[:, j], in_=io["wfcT8"][j])

    # evacuate V, free QKV PSUM banks
    for b in range(BPC):
        V_sb = sb.tile([128, D], BF16, tag="V_sb", name="V_sb")
        nc.vector.tensor_copy(out=V_sb, in_=Vp[b][:, :])
        qkv_sb[b][2] = V_sb
    ps_qkv.__exit__(None, None, None)

    # post-phase-A PSUM pools (2 banks each): VW accumulators, O banks, fc
    ps_vw = ctx.enter_context(tc.tile_pool(name="ps_vw", bufs=1, space="PSUM"))
    VWp = [ps_vw.tile([128, D], F32, tag=f"VWp{b}", name=f"VWp{b}") for b in range(BPC)]
    ps_ao = ctx.enter_context(tc.tile_pool(name="ps_ao", bufs=1, space="PSUM"))
    Obank = [ps_ao.tile([128, D], F32, tag=f"Ob{b}", name=f"Ob{b}") for b in range(BPC)]
    ps_fc = ctx.enter_context(tc.tile_pool(name="ps_fc", bufs=2, space="PSUM"))

    # ---- A@V for all heads ----
    Oscs = []
    for b in range(BPC):
        Osc = sb.tile([128, D], F32, tag="Osc", name="Osc")
        Oscs.append(Osc)
    for (b, h) in heads:
        nc.tensor.matmul(Obank[b][:, h * 64:(h + 1) * 64], aTs[(b, h)][:, :],
                         qkv_sb[b][2][:, h * 64:(h + 1) * 64], start=True, stop=True)
        nc.vector.tensor_scalar_mul(out=Oscs[b][:, h * 64:(h + 1) * 64],
                                    in0=Obank[b][:, h * 64:(h + 1) * 64],
                                    scalar1=rss[(b, h)])

    # ---- silu + LN + xT per batch (before VW so PE FIFO can't stall) ----
    xTs, xT8s, xhats = [], [], []
    for b in range(BPC):
        Osc = Oscs[b]
        sg = sb.tile([128, D], F32, tag="sg", name="sg")
        nc.scalar.activation(out=sg, in_=Osc, func=AF.Sigmoid)
        Osw = sb.tile([128, D], F32, tag="Osw", name="Osw")
        nc.vector.tensor_mul(out=Osw, in0=Osc, in1=sg)
        st6 = small.tile([128, 6], F32, tag="st6", name="st6")
        nc.vector.bn_stats(out=st6, in_=Osw)
        mv = small.tile([128, 2], F32, tag="mv", name="mv")
        nc.vector.bn_aggr(out=mv, in_=st6)
        sd = small.tile([128, 1], F32, tag="sd", name="sd")
        nc.scalar.activation(out=sd, in_=mv[:, 1:2], func=AF.Sqrt,
                             scale=float(D) / (D - 1))
        nc.vector.tensor_scalar_add(out=sd, in0=sd, scalar1=LN_EPS)
        rstd = small.tile([128, 1], F32, tag="rstd", name="rstd")
        nc.vector.reciprocal(rstd, sd)
        xhat = sb.tile([128, D], BF16, tag="xhat", name="xhat")
        nc.vector.tensor_scalar(out=xhat, in0=Osw, scalar1=mv[:, 0:1], scalar2=rstd,
                                op0=ALU.subtract, op1=ALU.mult)
        xT = sb.tile([128, 4, 128], BF16, tag="xT", name="xT")
        xT8 = sb.tile([128, 2, 2, 128], FP8, tag="xT8", name="xT8")
        for dc in range(4):
            tp = Tbank[:, tslot_next(), :]
            nc.tensor.transpose(tp, xhat[:, dc * 128:(dc + 1) * 128], ident[:, :])
            nc.vector.tensor_copy(out=xT[:, dc, :], in_=tp)
            nc.vector.tensor_copy(out=xT8[:, dc // 2, dc % 2, :], in_=tp)
        xTs.append(xT)
        xT8s.append(xT8)
        xhats.append(xhat)

    # ---- VW = (v @ wfc_eff) * SC_VW via fp8 DR, accumulating ----
    for pc in range(NC4):
        for b in range(BPC):
            nc.tensor.matmul(VWp[b][:, :], vT8_sb[b][:, pc, :, :], wfn_sb[:, pc, :, :],
                             start=pc == 0, stop=pc == NC4 - 1, perf_mode=DR)

    # ---- stats per batch ----
    cin_sb = small.tile([128, 2], F32, tag="cin_sb", name="cin_sb")
    s1l = [small.tile([128, 1], F32, tag=f"s1l{b}", name=f"s1l{b}") for b in range(BPC)]
    s2l = [small.tile([128, 1], F32, tag=f"s2l{b}", name=f"s2l{b}") for b in range(BPC)]
    for b in range(BPC):
        # stats: S1 = sv1 + xhat.wsum ; S2 = sv2 + (2/SC)xhat.VW + 2 xhat.vwc + xhat.(xhat G)
        # Zp reuses the (now idle) O bank so it doesn't couple into the fc
        # PSUM rotation and stall fc matmuls on the stats reads.
        Zp = Obank[b]
        a12 = Sbank[:, 3, 2 * b:2 * b + 2]
        xT = xTs[b]
        for dc in range(4):
            nc.tensor.matmul(Zp[:, :], xT[:, dc, :], G_sb[:, dc, :],
                             start=dc == 0, stop=dc == 3)
            nc.tensor.matmul(a12, xT[:, dc, :], wv2[:, dc, :],
                             start=dc == 0, stop=dc == 3)
        j1 = tpool.tile([128, D], BF16, tag="junk", name="junk1")
        j2 = tpool.tile([128, D], BF16, tag="junk", name="junk2")
        nc.vector.tensor_mul(out=j1, in0=xhats[b], in1=VWp[b][:, :])
        nc.vector.tensor_mul(out=j2, in0=xhats[b], in1=Zp[:, :])
        r1 = small.tile([128, 1], F32, tag="r1", name="r1")
        r2 = small.tile([128, 1], F32, tag="r2", name="r2")
        nc.vector.reduce_sum(r1, j1, axis=AX.X)
        nc.vector.reduce_sum(r2, j2, axis=AX.X)
        s2a = small.tile([128, 1], F32, tag="s2a", name="s2a")
        nc.vector.tensor_scalar(out=s2a, in0=r1, scalar1=2.0 / SC_VW,
                                scalar2=svin[:, 2 * b + 1:2 * b + 2],
                                op0=ALU.mult, op1=ALU.add)
        s2b = small.tile([128, 1], F32, tag="s2b", name="s2b")
        nc.vector.tensor_add(out=s2b, in0=s2a, in1=r2)
        nc.vector.tensor_add(out=s2l[b], in0=s2b, in1=a12[:, 1:2])
        nc.vector.tensor_add(out=s1l[b], in0=svin[:, 2 * b:2 * b + 1], in1=a12[:, 0:1])
    nc.vector.tensor_add(out=cin_sb[:, 0:1], in0=s1l[0], in1=s1l[1])
    nc.vector.tensor_add(out=cin_sb[:, 1:2], in0=s2l[0], in1=s2l[1])

    # ---- stats AllReduce (overlaps the fc phase below) ----
    cin = dram.tile([128, 2], F32, tag="cin", name="cin")
    cout = dram.tile([128, 2], F32, tag="cout", name="cout")
    nc.gpsimd.dma_start(out=cin[:, :], in_=cin_sb)
    if os.environ.get("V2_SKIP_COLL", "0") == "1":
        nc.gpsimd.dma_start(out=cout[:, :], in_=cin[:, :])
    else:
        nc.gpsimd.collective_compute(
            "AllReduce", ALU.add, replica_groups=[list(range(N_CORES))],
            ins=[cin.opt()], outs=[cout.opt()])
    red = small.tile([128, 2], F32, tag="red", name="red")
    nc.gpsimd.dma_start(out=red[:, :], in_=cout[:, :])

    # ---- fc matmuls (fp8 DR) + residual add (not AR-gated) ----
    tsegs = []
    for pt in range(8):
        for b in range(BPC):
            O2 = ps_fc.tile([128, 512], F32, tag="O2", name="O2")
            for j in range(2):
                nc.tensor.matmul(O2[:, :], xT8s[b][:, j, :, :],
                                 wfcT8[:, j, :, pt * 512:(pt + 1) * 512],
                                 start=j == 0, stop=j == 1, perf_mode=DR)
            tseg = tpool.tile([128, 512], BF16, tag="tseg", name="tseg")
            nc.vector.tensor_add(out=tseg, in0=veff_sb[b][:, pt * 512:(pt + 1) * 512],
                                 in1=O2[:, :])
            tsegs.append((b, pt, tseg))

    # ---- post-AR: BN affine factors (scl scaled by 1/SC_FC: the staged
    # tseg values are SC_FC * out_pre) ----
    inv_n = 1.0 / float(B * P)
    mean = small.tile([128, 1], F32, tag="mean", name="mean")
    nc.scalar.mul(out=mean, in_=red[:, 0:1], mul=inv_n)
    ex2 = small.tile([128, 1], F32, tag="ex2", name="ex2")
    nc.scalar.mul(out=ex2, in_=red[:, 1:2], mul=inv_n)
    msq = small.tile([128, 1], F32, tag="msq", name="msq")
    nc.vector.tensor_mul(out=msq, in0=mean, in1=mean)
    var = small.tile([128, 1], F32, tag="var", name="var")
    nc.vector.tensor_sub(out=var, in0=ex2, in1=msq)
    epsbn = consts.tile([128, 1], F32, tag="epsbn", name="epsbn")
    nc.vector.memset(epsbn, BN_EPS)
    sdv = small.tile([128, 1], F32, tag="sdv", name="sdv")
    nc.scalar.activation(out=sdv, in_=var, func=AF.Sqrt, bias=epsbn)
    invs = small.tile([128, 1], F32, tag="invs", name="invs")
    nc.vector.reciprocal(invs, sdv)
    scl = small.tile([128, 1], F32, tag="scl", name="scl")
    nc.vector.tensor_mul(out=scl, in0=bng, in1=invs)
    sclg = small.tile([128, 1], F32, tag="sclg", name="sclg")
    nc.vector.tensor_scalar_mul(out=sclg, in0=scl, scalar1=1.0 / SC_FC)
    tmp = small.tile([128, 1], F32, tag="tmp", name="tmp")
    nc.vector.tensor_mul(out=tmp, in0=mean, in1=scl)
    shf = small.tile([128, 1], F32, tag="shf", name="shf")
    nc.vector.tensor_sub(out=shf, in0=bnb, in1=tmp)

    # ---- AR-gated epilogue: scale+shift into per-batch staging tiles (ACT
    # handles even pt, DVE odd pt), then one big store per batch on separate
    # HWDGE queues (per-segment stores serialize ~1us each on one ring) ----
    stages = [stg.tile([128, P], BF16, tag=f"stage{b}", name=f"stage{b}")
              for b in range(BPC)]
    for i, (b, pt, tseg) in enumerate(tsegs):
        out_sl = stages[b][:, pt * 512:(pt + 1) * 512]
        if pt % 2 == 0:
            nc.scalar.activation(out=out_sl, in_=tseg, func=AF.Identity,
                                 bias=shf, scale=sclg)
        else:
            nc.vector.tensor_scalar(out=out_sl, in0=tseg, scalar1=sclg, scalar2=shf,
                                    op0=ALU.mult, op1=ALU.add)
    nc.sync.dma_start(out=io["out"][0], in_=stages[0][:, :])
    nc.scalar.dma_start(out=io["out"][1], in_=stages[1][:, :])


def _build(temp):
    key = (MODE, temp, _ENV_KEY())
    if key in _BUILD_CACHE:
        return _BUILD_CACHE[key]
    nc = bacc.Bacc("TRN2", target_bir_lowering=False, debug=False, num_devices=N_CORES)
    io = {
        "qT8": nc.dram_tensor("qT8", [BPC, 2, 128, 8, 2, 128], FP8, kind="ExternalInput").ap(),
        "kT8": nc.dram_tensor("kT8", [BPC, 2, 128, 8, 2, 128], FP8, kind="ExternalInput").ap(),
        "vT8": nc.dram_tensor("vT8", [BPC, 128, NC4, 2, 128], FP8, kind="ExternalInput").ap(),
        "vT16": nc.dram_tensor("vT16", [BPC, 128, 32, 128], BF16, kind="ExternalInput").ap(),
        "veff": nc.dram_tensor("veff", [BPC, C, P], BF16, kind="ExternalInput").ap(),
        "wq8": nc.dram_tensor("wq8", [4, 128, 4, 2, D], FP8, kind="ExternalInput").ap(),
        "wk8": nc.dram_tensor("wk8", [4, 128, 4, 2, D], FP8, kind="ExternalInput").ap(),
        "wv16": nc.dram_tensor("wv16", [4, 128, 8, D], BF16, kind="ExternalInput").ap(),
        "wfn": nc.dram_tensor("wfn", [128, NC4, 2, D], FP8, kind="ExternalInput").ap(),
        "wfcT8": nc.dram_tensor("wfcT8", [2, 128, 2, P], FP8, kind="ExternalInput").ap(),
        "G": nc.dram_tensor("G", [128, 4, D], BF16, kind="ExternalInput").ap(),
        "wv2": nc.dram_tensor("wv2", [128, 4, 2], BF16, kind="ExternalInput").ap(),
        "svin": nc.dram_tensor("svin", [C, 2 * BPC], F32, kind="ExternalInput").ap(),
        "bng": nc.dram_tensor("bng", [C, 1], F32, kind="ExternalInput").ap(),
        "bnb": nc.dram_tensor("bnb", [C, 1], F32, kind="ExternalInput").ap(),
        "out": nc.dram_tensor("out", [BPC, C, P], BF16, kind="ExternalOutput").ap(),
    }
    with tile.TileContext(nc) as tc, ExitStack() as ctx:
        _emit(ctx, nc, tc, io, temp)
    nc.compile()
    _BUILD_CACHE[key] = nc
    return nc


def _pack_dr_g(xT, ng, sub):
    # [P, M] -> [ng, 128, sub, 2, M]: group g, partition r, chunk sub, half i
    # covers row p = (g*sub + s)*256 + i*128 + r
    M = xT.shape[1]
    return np.ascontiguousarray(
        xT.reshape(ng, sub, 2, 128, M).transpose(0, 3, 1, 2, 4))


def _f8(x):
    return np.ascontiguousarray(np.asarray(x, np.float32).astype(ml_dtypes.float8_e4m3))


def _b16(x):
    return np.ascontiguousarray(np.asarray(x, np.float32).astype(ml_dtypes.bfloat16))


def kernel(v, k, q, w_qs, w_ks, w_vs, w_fc, ln_gamma, ln_beta, temperature,
           bn_gamma, bn_beta, **_ignored):
    v = np.asarray(v, np.float32)
    k = np.asarray(k, np.float32)
    q = np.asarray(q, np.float32)
    w_qs = np.asarray(w_qs, np.float32)
    w_ks = np.asarray(w_ks, np.float32)
    w_vs = np.asarray(w_vs, np.float32)
    w_fc = np.asarray(w_fc, np.float32)
    ln_gamma = np.asarray(ln_gamma, np.float32)
    ln_beta = np.asarray(ln_beta, np.float32)
    temp = float(np.asarray(temperature))
    bn_gamma = np.asarray(bn_gamma, np.float32)
    bn_beta = np.asarray(bn_beta, np.float32)

    qf = q.reshape(B, C, P)
    kf = k.reshape(B, C, P)
    vf = v.reshape(B, C, P)

    # activations: [P, C] transposed, group-DR-packed for q/k; r-major for vT8/vT16
    qT8 = np.stack([_f8(_pack_dr_g(qf[b].T, 2, 8)) for b in range(B)])
    kT8 = np.stack([_f8(_pack_dr_g(kf[b].T, 2, 8)) for b in range(B)])
    # vT8 SBUF layout [r, pc, i, c]:
    vT8 = np.stack([_f8(vf[b].T.reshape(NC4, 2, 128, C).transpose(2, 0, 1, 3))
                    for b in range(B)])
    # vT16 SBUF layout [r, pc2, c]:
    vT16 = np.stack([_b16(vf[b].T.reshape(32, 128, C).transpose(1, 0, 2))
                     for b in range(B)])

    # weights
    wq8 = _f8(_pack_dr_g(w_qs.T * SC_QK, 4, 4))
    wk8 = _f8(_pack_dr_g(w_ks.T * SC_QK, 4, 4))
    wv16 = _b16(w_vs.T.reshape(4, 8, 128, D).transpose(0, 2, 1, 3))
    wfc_eff = (w_fc * ln_gamma[None, :]).astype(np.float32)       # [P, D]
    # wfn SBUF layout [r, pc, i, d]:
    wfn8 = _f8((wfc_eff * SC_VW).reshape(NC4, 2, 128, D).transpose(2, 0, 1, 3))
    # wfcT8 [j, r, i, p]: d = j*256 + i*128 + r, pre-scaled by SC_FC
    wfcT8 = _f8((wfc_eff.T * SC_FC).reshape(2, 2, 128, P).transpose(0, 2, 1, 3))
    G = (wfc_eff.T.astype(np.float64) @ wfc_eff.astype(np.float64)).astype(np.float32)
    G16 = _b16(G.reshape(4, 128, D).transpose(1, 0, 2))           # [r, dc, d]
    bias_fc = (w_fc @ ln_beta).astype(np.float32)                 # [P]
    wsum = wfc_eff.sum(0)
    vwc = (bias_fc.astype(np.float64) @ wfc_eff.astype(np.float64)).astype(np.float32)
    wv2 = _b16(np.stack([wsum, 2.0 * vwc], axis=1)
               .reshape(4, 128, 2).transpose(1, 0, 2))            # [r, dc, j]

    veff_f = vf + bias_fc[None, None, :]
    veff = _b16(veff_f * SC_FC)     # device residual pre-scaled to match fp8 fc
    # per-batch input-residual partial sums for the BN stats decomposition
    sv1 = veff_f.astype(np.float64).sum(-1)                       # [B, C]
    sv2 = (veff_f.astype(np.float64) ** 2).sum(-1)
    svin_all = np.stack([sv1, sv2], axis=-1).astype(np.float32)   # [B, C, 2]

    bng = np.ascontiguousarray(bn_gamma.reshape(C, 1))
    bnb = np.ascontiguousarray(bn_beta.reshape(C, 1))

    nc = _build(temp)
    in_maps = []
    for i in range(N_CORES):
        bs = slice(BPC * i, BPC * (i + 1))
        # svin layout: [c, 2*b + (0:sum, 1:sumsq)]
        svin = np.ascontiguousarray(
            svin_all[bs].transpose(1, 0, 2).reshape(C, 2 * BPC))
        in_maps.append({
            "qT8": qT8[bs], "kT8": kT8[bs], "vT8": vT8[bs], "vT16": vT16[bs],
            "veff": veff[bs], "wq8": wq8, "wk8": wk8, "wv16": wv16,
            "wfn": wfn8, "wfcT8": wfcT8, "G": G16, "wv2": wv2,
            "svin": svin, "bng": bng, "bnb": bnb,
        })
    res = run_bass_kernel_spmd(nc, in_maps, core_ids=list(range(N_CORES)))
    global LAST_RESULTS
    LAST_RESULTS = res
    out = np.concatenate([np.asarray(res.results[i]["out"], np.float32)
                          for i in range(N_CORES)], axis=0)
    return out.reshape(B, C, HH, WW)
